# revision 9
# baseline (speedup 1.0000x reference)
"""Distributed Trainium2 kernel for nn_AdjEmbeddings (gnn_message_passing).

Strategy (8 NeuronCores, edge-sharded):
  Only ~E/NUM_USERS (~32) of the 3.2M edges match the single user_idx, so the
  only tensor that needs a full read is edge_src.  Per core (400k-edge shard):
    1. Stream the src shard [128,3125] and compare against user_idx (DVE).
    2. Block-summarize matches (blocks of 25 edges) -> [128,125] indicator.
    3. Per-partition top-2 matched-block extraction (reduce_max + clear).
    4. Indirect-DMA gather the <=2 matched blocks/partition from a host-packed
       [16000, 75] (src|dst|freq) array; re-mask; per-partition top-2 matched
       edges; unpack (dst, freq) from a packed value dst*64+freq.
    5. Indirect-DMA gather the matched POI embedding rows; PE matmuls produce
       [1, 128+1] = (partial numerator | partial denominator).
    6. AllGather[8,129] across the 8 cores; every core reduces the partials
       locally and computes the epilogue (neigh = num/max(den,1), fc matmuls).
  Unmatched gather slots point out-of-bounds (skipped by the DMA) and carry
  weight 0, so they contribute nothing regardless of sim/HW fill behavior.
  NOTE: same-engine RAW hazards are real on this HW -- every dependent DVE op
  is serialized through the vq semaphore.
"""
import sys

if '/opt/trn_rl_repo' not in sys.path:
    sys.path.insert(0, '/opt/trn_rl_repo')

import numpy as np

NCORES = 8
E = 3_200_000
ESH = E // NCORES            # 400_000 edges per core
P = 128
FREE = ESH // P              # 3125
BLK = 25                     # edges per summary block
NBLKF = FREE // BLK          # 125 blocks per partition
NBLK = ESH // BLK            # 16000 blocks per core
TOPK = 2                     # matched blocks / edges extracted per partition
DIM = 128
NPOI = 50_000
NUSR = 100_000
BLK_SENT = 20_000            # > NBLK-1  -> OOB, skipped
POI_SENT = 60_000            # > NPOI-1  -> OOB, skipped
CLEAR = 1.0e7                # subtracted to clear extracted maxima
CH0 = 1575                   # stream chunk split (multiple of BLK)
NB0 = CH0 // BLK


def _build2():
    from concourse import bass, mybir
    from contextlib import ExitStack

    nc = bass.Bass(num_devices=NCORES)
    f32, i32 = mybir.dt.float32, mybir.dt.int32
    Alu = mybir.AluOpType
    X = mybir.AxisListType.X

    src_in = nc.declare_dram_parameter("src", [P, FREE], i32, isOutput=False)
    packed_in = nc.declare_dram_parameter("packed", [NBLK, 3 * BLK], i32, isOutput=False)
    uidrep_in = nc.declare_dram_parameter("uidrep", [P, 1], f32, isOutput=False)
    uidpad_in = nc.declare_dram_parameter("uidpad", [2, 1], i32, isOutput=False)
    blkio_in = nc.declare_dram_parameter("blkio", [P, NBLKF], f32, isOutput=False)
    poi_in = nc.declare_dram_parameter("poi", [NPOI, DIM], f32, isOutput=False)
    uemb_in = nc.declare_dram_parameter("uemb", [NUSR, DIM], f32, isOutput=False)
    fcwt_in = nc.declare_dram_parameter("fcwt", [2 * DIM, DIM], f32, isOutput=False)
    fcb_in = nc.declare_dram_parameter("fcb", [1, DIM], f32, isOutput=False)
    out_ext = nc.declare_dram_parameter("out", [1, DIM], f32, isOutput=True)

    cc_in = nc.dram_tensor("cc_in", [1, DIM + 1], f32)
    cc_ag = nc.dram_tensor("cc_ag", [NCORES, DIM + 1], f32, addr_space="Shared")

    es = ExitStack()

    def sb(name, shape, dt):
        return es.enter_context(nc.sbuf_tensor(name, shape, dt))

    def ps(name, shape):
        return es.enter_context(nc.psum_tensor(name, shape, f32))

    src_sb = sb('src_sb', [P, FREE], i32)
    mask_sb = sb('mask_sb', [P, FREE], f32)
    summ_sb = sb('summ_sb', [P, NBLKF], f32)
    blkio_sb = sb('blkio_sb', [P, NBLKF], f32)
    cand_sb = sb('cand_sb', [P, NBLKF], f32)
    eqb_sb = sb('eqb_sb', [P, NBLKF], f32)
    mtop_sb = sb('mtop_sb', [P, TOPK], f32)
    mm_sb = sb('mm_sb', [P, TOPK], f32)
    mtmp_sb = sb('mtmp_sb', [P, TOPK], f32)
    moff_sb = sb('moff_sb', [P, TOPK], i32)
    uid_sb = sb('uid_sb', [P, 1], f32)
    upad_sb = sb('upad_sb', [2, 1], i32)
    warmoff_sb = sb('warmoff_sb', [2, 1], i32)
    warm_sb = sb('warm_sb', [2, 3 * BLK], i32)
    g_sb = sb('g_sb', [P, 3 * BLK * TOPK], i32)
    mask2_sb = sb('mask2_sb', [P, BLK * TOPK], f32)
    dstf_sb = sb('dstf_sb', [P, BLK * TOPK], f32)
    freqf_sb = sb('freqf_sb', [P, BLK * TOPK], f32)
    packf_sb = sb('packf_sb', [P, BLK * TOPK], f32)
    cand2_sb = sb('cand2_sb', [P, BLK * TOPK], f32)
    eq2_sb = sb('eq2_sb', [P, BLK * TOPK], f32)
    etop_sb = sb('etop_sb', [P, TOPK], f32)
    em_sb = sb('em_sb', [P, TOPK], f32)
    ei_sb = sb('ei_sb', [P, TOPK], i32)
    dsti_sb = sb('dsti_sb', [P, TOPK], i32)
    freqi_sb = sb('freqi_sb', [P, TOPK], i32)
    frf_sb = sb('frf_sb', [P, TOPK], f32)
    wf_sb = sb('wf_sb', [P, TOPK], f32)
    dstf2_sb = sb('dstf2_sb', [P, TOPK], f32)
    dstt_sb = sb('dstt_sb', [P, TOPK], f32)
    dstoff_sb = sb('dstoff_sb', [P, TOPK], i32)
    paug_sb = sb('paug_sb', [P, TOPK * (DIM + 1)], f32)
    u2_sb = sb('u2_sb', [2, DIM], f32)
    ucol_sb = sb('ucol_sb', [P, 1], f32)
    ncol_sb = sb('ncol_sb', [P, 1], f32)
    nd_sb = sb('nd_sb', [1, DIM + 1], f32)
    cc8_sb = sb('cc8_sb', [NCORES, DIM + 1], f32)
    ones8_sb = sb('ones8_sb', [NCORES, 1], f32)
    saf_sb = sb('saf_sb', [1, 1], f32)
    rs_sb = sb('rs_sb', [1, 1], f32)
    t1_sb = sb('t1_sb', [1, DIM], f32)
    t2_sb = sb('t2_sb', [1, DIM], f32)
    fcw1_sb = sb('fcw1_sb', [P, DIM], f32)
    fcw2_sb = sb('fcw2_sb', [P, DIM], f32)
    fcb_sb = sb('fcb_sb', [1, DIM], f32)
    out_sb = sb('out_sb', [1, DIM], f32)
    ones11_sb = sb('ones11_sb', [1, 1], f32)

    psum_t1 = ps('psum_t1', [P, 1])
    psum_fc1 = ps('psum_fc1', [1, DIM])
    psum_nd = ps('psum_nd', [1, DIM + 1])
    psum_fc2 = ps('psum_fc2', [1, DIM])
    psum_nc = ps('psum_nc', [P, 1])
    psum_den = ps('psum_den', [1, 1])

    MK = {}
    with (
        nc.semaphore("vq") as vq,
        nc.semaphore("sS0") as sS0,
        nc.semaphore("sS1") as sS1,
        nc.semaphore("sC") as sC,
        nc.semaphore("sG") as sG,
        nc.semaphore("sPE") as sPE,
        nc.semaphore("sCCI") as sCCI,
        nc.semaphore("sCC") as sCC,
        nc.semaphore("sRED") as sRED,
        nc.Block() as block,
    ):
        @block.vector
        def _(vector):
            v = nc.vector
            nv = [0]

            def step(inst, wait=True):
                inst.then_inc(vq, 1)
                nv[0] += 1
                # serialize same-engine RAW hazards; independent ops may skip
                if wait:
                    vector.wait_ge(vq, nv[0])
                return nv[0]

            # independent setup (no internal deps -> no waits between them)
            step(v.memset(warmoff_sb[:], 0), wait=False)
            step(v.memset(ones11_sb[:], 1.0), wait=False)
            step(v.memset(g_sb[:], -1), wait=False)
            step(v.memset(paug_sb[:], 0.0), wait=False)
            step(v.memset(
                paug_sb[:].rearrange("p (j c) -> p j c", c=DIM + 1)[:, :, DIM:DIM + 1],
                1.0), wait=False)
            step(v.memset(ones8_sb[:], 1.0), wait=False)
            MK['setup'] = nv[0]
            vector.wait_ge(vq, nv[0])
            vector.wait_ge(sC, 80)
            vector.wait_ge(sS0, 32)     # blkio + src chunk0
            step(v.tensor_scalar(out=mask_sb[:, 0:CH0], in0=src_sb[:, 0:CH0],
                                 scalar1=uid_sb[:, :1], scalar2=None, op0=Alu.is_equal))
            step(v.tensor_reduce(
                out=summ_sb[:, 0:NB0],
                in_=mask_sb[:, 0:CH0].rearrange("p (b w) -> p b w", w=BLK),
                axis=X, op=Alu.max))
            vector.wait_ge(sS1, 16)
            step(v.tensor_scalar(out=mask_sb[:, CH0:FREE], in0=src_sb[:, CH0:FREE],
                                 scalar1=uid_sb[:, :1], scalar2=None, op0=Alu.is_equal))
            step(v.tensor_reduce(
                out=summ_sb[:, NB0:NBLKF],
                in_=mask_sb[:, CH0:FREE].rearrange("p (b w) -> p b w", w=BLK),
                axis=X, op=Alu.max))
            # cand = summ * (blkid+1) - 1   (blkio holds blkid+1)
            step(v.tensor_tensor(out=cand_sb[:], in0=summ_sb[:], in1=blkio_sb[:],
                                 op=Alu.mult))
            step(v.tensor_scalar_add(out=cand_sb[:], in0=cand_sb[:], scalar1=-1.0))
            # top-2 blocks per partition
            step(v.tensor_reduce(out=mtop_sb[:, 0:1], in_=cand_sb[:], axis=X, op=Alu.max))
            step(v.tensor_scalar(out=eqb_sb[:], in0=cand_sb[:],
                                 scalar1=mtop_sb[:, 0:1], scalar2=CLEAR,
                                 op0=Alu.is_equal, op1=Alu.mult))
            step(v.tensor_tensor(out=cand_sb[:], in0=cand_sb[:], in1=eqb_sb[:],
                                 op=Alu.subtract))
            step(v.tensor_reduce(out=mtop_sb[:, 1:2], in_=cand_sb[:], axis=X, op=Alu.max))
            # moff = matched ? blkid : BLK_SENT  (mtop holds blkid, or < 0)
            step(v.tensor_scalar(out=mm_sb[:], in0=mtop_sb[:], scalar1=0.0,
                                 scalar2=None, op0=Alu.is_ge))
            step(v.scalar_tensor_tensor(out=mtmp_sb[:], in0=mtop_sb[:],
                                        scalar=-float(BLK_SENT), in1=mm_sb[:],
                                        op0=Alu.add, op1=Alu.mult))
            step(v.tensor_scalar(out=moff_sb[:], in0=mtmp_sb[:],
                                 scalar1=float(BLK_SENT), scalar2=None, op0=Alu.add))
            MK['moff'] = nv[0]
            # ---- level 2: gathered blocks -> matched edges
            vector.wait_ge(sG, 64)          # warm(16)+u(16)+2 block gathers
            g3 = g_sb[:].rearrange("p (j c) -> p j c", c=3 * BLK)
            m23 = mask2_sb[:].rearrange("p (j c) -> p j c", c=BLK)
            d3 = dstf_sb[:].rearrange("p (j c) -> p j c", c=BLK)
            f3 = freqf_sb[:].rearrange("p (j c) -> p j c", c=BLK)
            step(v.tensor_scalar(out=m23, in0=g3[:, :, 0:BLK], scalar1=uid_sb[:, :1],
                                 scalar2=None, op0=Alu.is_equal), wait=False)
            step(v.tensor_copy(out=d3, in_=g3[:, :, BLK:2 * BLK]), wait=False)
            step(v.tensor_copy(out=f3, in_=g3[:, :, 2 * BLK:3 * BLK]))
            # packf = dst*64 + freq ; cand2 = (packf+1)*mask2 - 1
            step(v.scalar_tensor_tensor(out=packf_sb[:], in0=dstf_sb[:], scalar=64.0,
                                        in1=freqf_sb[:], op0=Alu.mult, op1=Alu.add))
            step(v.scalar_tensor_tensor(out=cand2_sb[:], in0=packf_sb[:], scalar=1.0,
                                        in1=mask2_sb[:], op0=Alu.add, op1=Alu.mult))
            step(v.tensor_scalar_add(out=cand2_sb[:], in0=cand2_sb[:], scalar1=-1.0))
            step(v.tensor_reduce(out=etop_sb[:, 0:1], in_=cand2_sb[:], axis=X, op=Alu.max))
            step(v.tensor_scalar(out=eq2_sb[:], in0=cand2_sb[:],
                                 scalar1=etop_sb[:, 0:1], scalar2=CLEAR,
                                 op0=Alu.is_equal, op1=Alu.mult))
            step(v.tensor_tensor(out=cand2_sb[:], in0=cand2_sb[:], in1=eq2_sb[:],
                                 op=Alu.subtract))
            step(v.tensor_reduce(out=etop_sb[:, 1:2], in_=cand2_sb[:], axis=X, op=Alu.max))
            # unpack: etop = dst*64+freq (>=64) matched, else < 0
            step(v.tensor_scalar(out=em_sb[:], in0=etop_sb[:], scalar1=0.0,
                                 scalar2=None, op0=Alu.is_ge))
            step(v.tensor_copy(out=ei_sb[:], in_=etop_sb[:]))
            step(v.tensor_scalar(out=dsti_sb[:], in0=ei_sb[:], scalar1=6, scalar2=None,
                                 op0=Alu.arith_shift_right), wait=False)
            step(v.tensor_scalar(out=freqi_sb[:], in0=ei_sb[:], scalar1=63, scalar2=None,
                                 op0=Alu.bitwise_and))
            step(v.tensor_copy(out=frf_sb[:], in_=freqi_sb[:]), wait=False)
            step(v.tensor_copy(out=dstf2_sb[:], in_=dsti_sb[:]))
            step(v.tensor_tensor(out=wf_sb[:], in0=frf_sb[:], in1=em_sb[:],
                                 op=Alu.mult), wait=False)
            step(v.scalar_tensor_tensor(out=dstt_sb[:], in0=dstf2_sb[:],
                                        scalar=-float(POI_SENT), in1=em_sb[:],
                                        op0=Alu.add, op1=Alu.mult))
            step(v.tensor_scalar(out=dstoff_sb[:], in0=dstt_sb[:],
                                 scalar1=float(POI_SENT), scalar2=None, op0=Alu.add))
            MK['dstoff'] = nv[0]
            # u column for the fc matmul (PE transposed it into psum_t1)
            vector.wait_ge(sPE, 1)
            step(v.tensor_copy(out=ucol_sb[:], in_=psum_t1[:]))
            MK['ucol'] = nv[0]
            # partials out for the collective
            vector.wait_ge(sPE, 3)
            step(v.tensor_copy(out=nd_sb[:], in_=psum_nd[:]))
            MK['nd'] = nv[0]
            # ---- after allgather: PE summed the partials into psum_nc/psum_den
            vector.wait_ge(sPE, 4)
            step(v.tensor_copy(out=ncol_sb[:], in_=psum_nc[:]), wait=False)
            # den is 0 (no matches anywhere -> num==0) or >= 1
            step(v.tensor_scalar(out=saf_sb[:], in0=psum_den[:], scalar1=1.0,
                                 scalar2=None, op0=Alu.max))
            MK['ncol'] = nv[0]
            step(v.reciprocal(out=rs_sb[:], in_=saf_sb[:]))
            MK['rs'] = nv[0]
            vector.wait_ge(sPE, 5)
            step(v.tensor_scalar(out=t1_sb[:], in0=psum_fc2[:], scalar1=rs_sb[0:1, :1],
                                 scalar2=None, op0=Alu.mult))
            step(v.tensor_tensor(out=t2_sb[:], in0=t1_sb[:], in1=psum_fc1[:], op=Alu.add))
            step(v.tensor_tensor(out=out_sb[:], in0=t2_sb[:], in1=fcb_sb[:], op=Alu.add))
            MK['out'] = nv[0]

        @block.sync
        def _(sync):
            sync.dma_start(out=uid_sb[:], in_=uidrep_in[:]).then_inc(sC, 16)
            sync.dma_start(out=upad_sb[:], in_=uidpad_in[:]).then_inc(sC, 16)
            sync.dma_start(out=fcb_sb[:], in_=fcb_in[:]).then_inc(sC, 16)
            sync.dma_start(out=fcw1_sb[:], in_=fcwt_in[0:DIM, :]).then_inc(sC, 16)
            sync.dma_start(out=fcw2_sb[:], in_=fcwt_in[DIM:2 * DIM, :]).then_inc(sC, 16)
            sync.dma_start(out=blkio_sb[:], in_=blkio_in[:]).then_inc(sS0, 16)
            sync.wait_ge(vq, MK['nd'])
            sync.dma_start(out=cc_in[:], in_=nd_sb[:]).then_inc(sCCI, 16)
            sync.wait_ge(sCC, 1)
            sync.dma_start(out=cc8_sb[:], in_=cc_ag[:]).then_inc(sRED, 16)
            sync.wait_ge(vq, MK['out'])
            sync.dma_start(out=out_ext[:], in_=out_sb[:]).then_inc(sS0, 16)

        @block.scalar
        def _(scalar):
            # second HWDGE ring: the big src stream
            scalar.dma_start(out=src_sb[:, 0:CH0], in_=src_in[:, 0:CH0]).then_inc(sS0, 16)
            scalar.dma_start(out=src_sb[:, CH0:FREE], in_=src_in[:, CH0:FREE]).then_inc(sS1, 16)

        @block.gpsimd
        def _(gpsimd):
            # warmup: pulls the indirect-DMA ucode load off the critical path
            gpsimd.wait_ge(vq, MK['setup'])
            gpsimd.indirect_dma_start(
                out=warm_sb[:], out_offset=None, in_=packed_in[:],
                in_offset=bass.IndirectOffsetOnAxis(ap=warmoff_sb[:, :1], axis=0),
                bounds_check=NBLK - 1, oob_is_err=False).then_inc(sG, 16)
            gpsimd.wait_ge(sC, 80)
            gpsimd.indirect_dma_start(
                out=u2_sb[:], out_offset=None, in_=uemb_in[:],
                in_offset=bass.IndirectOffsetOnAxis(ap=upad_sb[:, :1], axis=0),
                bounds_check=NUSR - 1, oob_is_err=False).then_inc(sG, 16)
            gpsimd.wait_ge(vq, MK['moff'])
            for j in range(TOPK):
                gpsimd.indirect_dma_start(
                    out=g_sb[:, j * 3 * BLK:(j + 1) * 3 * BLK], out_offset=None,
                    in_=packed_in[:],
                    in_offset=bass.IndirectOffsetOnAxis(ap=moff_sb[:, j:j + 1], axis=0),
                    bounds_check=NBLK - 1, oob_is_err=False).then_inc(sG, 16)
            gpsimd.wait_ge(vq, MK['dstoff'])
            for j in range(TOPK):
                gpsimd.indirect_dma_start(
                    out=paug_sb[:, j * (DIM + 1):j * (DIM + 1) + DIM], out_offset=None,
                    in_=poi_in[:],
                    in_offset=bass.IndirectOffsetOnAxis(ap=dstoff_sb[:, j:j + 1], axis=0),
                    bounds_check=NPOI - 1, oob_is_err=False).then_inc(sG, 16)
            gpsimd.wait_ge(sCCI, 16)
            gpsimd.collective_compute(
                "AllGather", mybir.AluOpType.bypass,
                replica_groups=[list(range(NCORES))],
                ins=[cc_in[:]], outs=[cc_ag[:]]).then_inc(sCC, 1)

        @block.tensor
        def _(tensor):
            tensor.wait_ge(sG, 32)            # u2 gathered
            tensor.wait_ge(vq, MK['setup'])   # ones11
            nc.tensor.transpose(out=psum_t1[:], in_=u2_sb[0:1, :],
                                identity=ones11_sb[:]).then_inc(sPE, 1)
            tensor.wait_ge(vq, MK['ucol'])
            tensor.wait_ge(sC, 80)
            nc.tensor.matmul(out=psum_fc1[:], lhsT=ucol_sb[:], rhs=fcw1_sb[:],
                             start=True, stop=True).then_inc(sPE, 1)
            tensor.wait_ge(vq, MK['dstoff'])
            tensor.wait_ge(sG, 96)            # poi gathered
            for j in range(TOPK):
                mmx = nc.tensor.matmul(
                    out=psum_nd[:], lhsT=wf_sb[:, j:j + 1],
                    rhs=paug_sb[:, j * (DIM + 1):(j + 1) * (DIM + 1)],
                    start=(j == 0), stop=(j == TOPK - 1))
            mmx.then_inc(sPE, 1)
            tensor.wait_ge(sRED, 16)
            nc.tensor.matmul(out=psum_nc[:], lhsT=cc8_sb[:, 0:DIM], rhs=ones8_sb[:],
                             start=True, stop=True)
            nc.tensor.matmul(out=psum_den[:], lhsT=cc8_sb[:, DIM:DIM + 1],
                             rhs=ones8_sb[:], start=True, stop=True).then_inc(sPE, 1)
            tensor.wait_ge(vq, MK['ncol'])
            nc.tensor.matmul(out=psum_fc2[:], lhsT=ncol_sb[:], rhs=fcw2_sb[:],
                             start=True, stop=True).then_inc(sPE, 1)

    es.close()
    return nc




def _build1():
    """TOPK=1 fast path: at most one matched edge per partition (asserted on
    the host). mask+candidate fused into one pass over an edge-index iota;
    single reduce_max; gather (dst,freq) pairs directly; no block level."""
    from concourse import bass, mybir
    from contextlib import ExitStack

    nc = bass.Bass(num_devices=NCORES)
    f32, i32 = mybir.dt.float32, mybir.dt.int32
    Alu = mybir.AluOpType
    X = mybir.AxisListType.X
    ESENT = 1_000_000            # > ESH-1 -> OOB, skipped

    src_in = nc.declare_dram_parameter("src", [P, FREE], i32, isOutput=False)
    dfpk_in = nc.declare_dram_parameter("dfpk", [ESH, 2], i32, isOutput=False)
    uidrep_in = nc.declare_dram_parameter("uidrep", [P, 1], f32, isOutput=False)
    uidpad_in = nc.declare_dram_parameter("uidpad", [2, 1], i32, isOutput=False)
    fiota_in = nc.declare_dram_parameter("fiota", [P, FREE], f32, isOutput=False)
    pbase_in = nc.declare_dram_parameter("pbase", [P, 1], f32, isOutput=False)
    poi_in = nc.declare_dram_parameter("poi", [NPOI, DIM], f32, isOutput=False)
    uemb_in = nc.declare_dram_parameter("uemb", [NUSR, DIM], f32, isOutput=False)
    fcwt_in = nc.declare_dram_parameter("fcwt", [2 * DIM, DIM], f32, isOutput=False)
    fcb_in = nc.declare_dram_parameter("fcb", [1, DIM], f32, isOutput=False)
    out_ext = nc.declare_dram_parameter("out", [1, DIM], f32, isOutput=True)

    cc_in = nc.dram_tensor("cc_in", [1, DIM + 1], f32)
    cc_ag = nc.dram_tensor("cc_ag", [NCORES, DIM + 1], f32, addr_space="Shared")

    es = ExitStack()

    def sb(name, shape, dt):
        return es.enter_context(nc.sbuf_tensor(name, shape, dt))

    def ps(name, shape):
        return es.enter_context(nc.psum_tensor(name, shape, f32))

    src_sb = sb('src_sb', [P, FREE], i32)
    fiota_sb = sb('fiota_sb', [P, FREE], f32)
    cand_sb = sb('cand_sb', [P, FREE], f32)
    ftop_sb = sb('ftop_sb', [P, 1], f32)
    m_sb = sb('m_sb', [P, 1], f32)
    pbase_sb = sb('pbase_sb', [P, 1], f32)
    t0_sb = sb('t0_sb', [P, 1], f32)
    t3_sb = sb('t3_sb', [P, 1], f32)
    eoff_sb = sb('eoff_sb', [P, 1], i32)
    g2_sb = sb('g2_sb', [P, 2], i32)
    frf_sb = sb('frf_sb', [P, 1], f32)
    wf_sb = sb('wf_sb', [P, 1], f32)
    dstf_sb = sb('dstf_sb', [P, 1], f32)
    dstt_sb = sb('dstt_sb', [P, 1], f32)
    poff_sb = sb('poff_sb', [P, 1], i32)
    paug_sb = sb('paug_sb', [P, DIM + 1], f32)
    uid_sb = sb('uid_sb', [P, 1], f32)
    upad_sb = sb('upad_sb', [2, 1], i32)
    warmoff_sb = sb('warmoff_sb', [2, 1], i32)
    warm_sb = sb('warm_sb', [2, 2], i32)
    u2_sb = sb('u2_sb', [2, DIM], f32)
    ucol_sb = sb('ucol_sb', [P, 1], f32)
    ncol_sb = sb('ncol_sb', [P, 1], f32)
    nd_sb = sb('nd_sb', [1, DIM + 1], f32)
    cc8_sb = sb('cc8_sb', [NCORES, DIM + 1], f32)
    ones8_sb = sb('ones8_sb', [NCORES, 1], f32)
    saf_sb = sb('saf_sb', [1, 1], f32)
    rs_sb = sb('rs_sb', [1, 1], f32)
    t1_sb = sb('t1_sb', [1, DIM], f32)
    t2_sb = sb('t2_sb', [1, DIM], f32)
    fcw1_sb = sb('fcw1_sb', [P, DIM], f32)
    fcw2_sb = sb('fcw2_sb', [P, DIM], f32)
    fcb_sb = sb('fcb_sb', [1, DIM], f32)
    out_sb = sb('out_sb', [1, DIM], f32)
    ones11_sb = sb('ones11_sb', [1, 1], f32)

    psum_t1 = ps('psum_t1', [P, 1])
    psum_fc1 = ps('psum_fc1', [1, DIM])
    psum_nd = ps('psum_nd', [1, DIM + 1])
    psum_fc2 = ps('psum_fc2', [1, DIM])
    psum_nc = ps('psum_nc', [P, 1])
    psum_den = ps('psum_den', [1, 1])

    MK = {}
    with (
        nc.semaphore("vq") as vq,
        nc.semaphore("sS0") as sS0,
        nc.semaphore("sS1") as sS1,
        nc.semaphore("sC") as sC,
        nc.semaphore("sG") as sG,
        nc.semaphore("sPE") as sPE,
        nc.semaphore("sCCI") as sCCI,
        nc.semaphore("sCC") as sCC,
        nc.semaphore("sRED") as sRED,
        nc.Block() as block,
    ):
        @block.vector
        def _(vector):
            v = nc.vector
            nv = [0]

            def step(inst, wait=True):
                inst.then_inc(vq, 1)
                nv[0] += 1
                if wait:
                    vector.wait_ge(vq, nv[0])
                return nv[0]

            step(v.memset(warmoff_sb[:], 0), wait=False)
            step(v.memset(ones11_sb[:], 1.0), wait=False)
            step(v.memset(g2_sb[:], 0), wait=False)
            step(v.memset(paug_sb[:, 0:DIM], 0.0), wait=False)
            step(v.memset(paug_sb[:, DIM:DIM + 1], 1.0), wait=False)
            step(v.memset(ones8_sb[:], 1.0), wait=False)
            MK['setup'] = nv[0]
            vector.wait_ge(vq, nv[0])
            vector.wait_ge(sC, 96)
            vector.wait_ge(sS0, 32)     # fiota + src chunk0
            # cand = (src == uid) * (f+1)   -- one fused pass per chunk
            step(v.scalar_tensor_tensor(out=cand_sb[:, 0:CH0], in0=src_sb[:, 0:CH0],
                                        scalar=uid_sb[:, :1], in1=fiota_sb[:, 0:CH0],
                                        op0=Alu.is_equal, op1=Alu.mult))
            vector.wait_ge(sS1, 16)
            step(v.scalar_tensor_tensor(out=cand_sb[:, CH0:FREE], in0=src_sb[:, CH0:FREE],
                                        scalar=uid_sb[:, :1], in1=fiota_sb[:, CH0:FREE],
                                        op0=Alu.is_equal, op1=Alu.mult))
            step(v.tensor_reduce(out=ftop_sb[:], in_=cand_sb[:], axis=X, op=Alu.max))
            # ftop = f+1 (>=1) if matched else 0
            step(v.tensor_scalar(out=m_sb[:], in0=ftop_sb[:], scalar1=0.0,
                                 scalar2=None, op0=Alu.is_gt))
            # edge offset = (ftop-1+pbase - ESENT)*m + ESENT
            step(v.scalar_tensor_tensor(out=t0_sb[:], in0=ftop_sb[:],
                                        scalar=-1.0 - ESENT, in1=pbase_sb[:],
                                        op0=Alu.add, op1=Alu.add))
            step(v.tensor_tensor(out=t3_sb[:], in0=t0_sb[:], in1=m_sb[:], op=Alu.mult))
            step(v.tensor_scalar(out=eoff_sb[:], in0=t3_sb[:], scalar1=float(ESENT),
                                 scalar2=None, op0=Alu.add))
            MK['eoff'] = nv[0]
            # ---- after (dst,freq) gather
            vector.wait_ge(sG, 48)
            step(v.tensor_copy(out=frf_sb[:], in_=g2_sb[:, 1:2]), wait=False)
            step(v.tensor_copy(out=dstf_sb[:], in_=g2_sb[:, 0:1]))
            step(v.tensor_tensor(out=wf_sb[:], in0=frf_sb[:], in1=m_sb[:],
                                 op=Alu.mult), wait=False)
            step(v.scalar_tensor_tensor(out=dstt_sb[:], in0=dstf_sb[:],
                                        scalar=-float(POI_SENT), in1=m_sb[:],
                                        op0=Alu.add, op1=Alu.mult))
            step(v.tensor_scalar(out=poff_sb[:], in0=dstt_sb[:],
                                 scalar1=float(POI_SENT), scalar2=None, op0=Alu.add))
            MK['poff'] = nv[0]
            vector.wait_ge(sPE, 1)
            step(v.tensor_copy(out=ucol_sb[:], in_=psum_t1[:]))
            MK['ucol'] = nv[0]
            vector.wait_ge(sPE, 3)
            step(v.tensor_copy(out=nd_sb[:], in_=psum_nd[:]))
            MK['nd'] = nv[0]
            # ---- after allgather: PE summed partials into psum_nc/psum_den
            vector.wait_ge(sPE, 4)
            step(v.tensor_copy(out=ncol_sb[:], in_=psum_nc[:]), wait=False)
            step(v.tensor_scalar(out=saf_sb[:], in0=psum_den[:], scalar1=1.0,
                                 scalar2=None, op0=Alu.max))
            MK['ncol'] = nv[0]
            step(v.reciprocal(out=rs_sb[:], in_=saf_sb[:]))
            vector.wait_ge(sPE, 5)
            step(v.tensor_scalar(out=t1_sb[:], in0=psum_fc2[:], scalar1=rs_sb[0:1, :1],
                                 scalar2=None, op0=Alu.mult))
            step(v.tensor_tensor(out=t2_sb[:], in0=t1_sb[:], in1=psum_fc1[:], op=Alu.add))
            step(v.tensor_tensor(out=out_sb[:], in0=t2_sb[:], in1=fcb_sb[:], op=Alu.add))
            MK['out'] = nv[0]

        @block.sync
        def _(sync):
            sync.dma_start(out=uid_sb[:], in_=uidrep_in[:]).then_inc(sC, 16)
            sync.dma_start(out=upad_sb[:], in_=uidpad_in[:]).then_inc(sC, 16)
            sync.dma_start(out=fcb_sb[:], in_=fcb_in[:]).then_inc(sC, 16)
            sync.dma_start(out=pbase_sb[:], in_=pbase_in[:]).then_inc(sC, 16)
            sync.dma_start(out=fcw1_sb[:], in_=fcwt_in[0:DIM, :]).then_inc(sC, 16)
            sync.dma_start(out=fcw2_sb[:], in_=fcwt_in[DIM:2 * DIM, :]).then_inc(sC, 16)
            sync.wait_ge(vq, MK['nd'])
            sync.dma_start(out=cc_in[:], in_=nd_sb[:]).then_inc(sCCI, 16)
            sync.wait_ge(sCC, 1)
            sync.dma_start(out=cc8_sb[:], in_=cc_ag[:]).then_inc(sRED, 16)
            sync.wait_ge(vq, MK['out'])
            sync.dma_start(out=out_ext[:], in_=out_sb[:]).then_inc(sS0, 16)

        @block.scalar
        def _(scalar):
            scalar.dma_start(out=fiota_sb[:], in_=fiota_in[:]).then_inc(sS0, 16)
            scalar.dma_start(out=src_sb[:, 0:CH0], in_=src_in[:, 0:CH0]).then_inc(sS0, 16)
            scalar.dma_start(out=src_sb[:, CH0:FREE], in_=src_in[:, CH0:FREE]).then_inc(sS1, 16)

        @block.gpsimd
        def _(gpsimd):
            gpsimd.wait_ge(vq, MK['setup'])
            gpsimd.indirect_dma_start(
                out=warm_sb[:], out_offset=None, in_=dfpk_in[:],
                in_offset=bass.IndirectOffsetOnAxis(ap=warmoff_sb[:, :1], axis=0),
                bounds_check=ESH - 1, oob_is_err=False).then_inc(sG, 16)
            gpsimd.wait_ge(sC, 96)
            gpsimd.indirect_dma_start(
                out=u2_sb[:], out_offset=None, in_=uemb_in[:],
                in_offset=bass.IndirectOffsetOnAxis(ap=upad_sb[:, :1], axis=0),
                bounds_check=NUSR - 1, oob_is_err=False).then_inc(sG, 16)
            gpsimd.wait_ge(vq, MK['eoff'])
            gpsimd.indirect_dma_start(
                out=g2_sb[:], out_offset=None, in_=dfpk_in[:],
                in_offset=bass.IndirectOffsetOnAxis(ap=eoff_sb[:, :1], axis=0),
                bounds_check=ESH - 1, oob_is_err=False).then_inc(sG, 16)
            gpsimd.wait_ge(vq, MK['poff'])
            gpsimd.indirect_dma_start(
                out=paug_sb[:, 0:DIM], out_offset=None, in_=poi_in[:],
                in_offset=bass.IndirectOffsetOnAxis(ap=poff_sb[:, :1], axis=0),
                bounds_check=NPOI - 1, oob_is_err=False).then_inc(sG, 16)
            gpsimd.wait_ge(sCCI, 16)
            gpsimd.collective_compute(
                "AllGather", mybir.AluOpType.bypass,
                replica_groups=[list(range(NCORES))],
                ins=[cc_in[:]], outs=[cc_ag[:]]).then_inc(sCC, 1)

        @block.tensor
        def _(tensor):
            tensor.wait_ge(sG, 32)
            tensor.wait_ge(vq, MK['setup'])
            nc.tensor.transpose(out=psum_t1[:], in_=u2_sb[0:1, :],
                                identity=ones11_sb[:]).then_inc(sPE, 1)
            tensor.wait_ge(vq, MK['ucol'])
            tensor.wait_ge(sC, 96)
            nc.tensor.matmul(out=psum_fc1[:], lhsT=ucol_sb[:], rhs=fcw1_sb[:],
                             start=True, stop=True).then_inc(sPE, 1)
            tensor.wait_ge(vq, MK['poff'])
            tensor.wait_ge(sG, 64)
            nc.tensor.matmul(out=psum_nd[:], lhsT=wf_sb[:], rhs=paug_sb[:],
                             start=True, stop=True).then_inc(sPE, 1)
            tensor.wait_ge(sRED, 16)
            nc.tensor.matmul(out=psum_nc[:], lhsT=cc8_sb[:, 0:DIM], rhs=ones8_sb[:],
                             start=True, stop=True)
            nc.tensor.matmul(out=psum_den[:], lhsT=cc8_sb[:, DIM:DIM + 1],
                             rhs=ones8_sb[:], start=True, stop=True).then_inc(sPE, 1)
            tensor.wait_ge(vq, MK['ncol'])
            nc.tensor.matmul(out=psum_fc2[:], lhsT=ncol_sb[:], rhs=fcw2_sb[:],
                             start=True, stop=True).then_inc(sPE, 1)

    es.close()
    return nc


_BUILT = {}


def _get_nc(path=1):
    if path not in _BUILT:
        _BUILT[path] = _build1() if path == 1 else _build2()
    return _BUILT[path]


_BLKIO = None


def _make_in_maps(inputs):
    global _BLKIO
    user_idx = np.asarray(inputs["user_idx"]).astype(np.int32)
    poi = np.ascontiguousarray(np.asarray(inputs["poi_embeddings"], dtype=np.float32))
    src = np.asarray(inputs["edge_src"]).astype(np.int32)
    dst = np.asarray(inputs["edge_dst"]).astype(np.int32)
    freq = np.asarray(inputs["edge_freq"]).astype(np.int32)
    uemb = np.ascontiguousarray(np.asarray(inputs["user_emb"], dtype=np.float32))
    fc_w = np.asarray(inputs["fc_w"], dtype=np.float32)
    fc_b = np.asarray(inputs["fc_b"], dtype=np.float32)

    uid = int(user_idx[0])
    uidrep = np.full((P, 1), float(uid), np.float32)
    uidpad = np.full((2, 1), uid, np.int32)
    fcwt = np.ascontiguousarray(fc_w.T)
    fcb = fc_b.reshape(1, DIM)
    m = src == uid
    mpart = m.reshape(NCORES * P, FREE)
    epp = mpart.sum(1)
    path = 1 if epp.max() <= 1 else 2

    in_maps = []
    if path == 1:
        fiota = np.broadcast_to(np.arange(1, FREE + 1, dtype=np.float32), (P, FREE))
        fiota = np.ascontiguousarray(fiota)
        pbase = (np.arange(P, dtype=np.float32) * FREE).reshape(P, 1)
        for c in range(NCORES):
            sl = slice(c * ESH, (c + 1) * ESH)
            dfpk = np.ascontiguousarray(
                np.stack([dst[sl], freq[sl]], axis=1))
            in_maps.append({
                "src": np.ascontiguousarray(src[sl].reshape(P, FREE)),
                "dfpk": dfpk, "uidrep": uidrep, "uidpad": uidpad,
                "fiota": fiota, "pbase": pbase,
                "poi": poi, "uemb": uemb, "fcwt": fcwt, "fcb": fcb,
            })
        return path, in_maps

    # TOPK=2 fallback: verify the static graph capacity (fail loudly
    # rather than return a wrong answer).
    if _BLKIO is None:
        _BLKIO = (np.arange(P * NBLKF, dtype=np.float32) + 1.0).reshape(P, NBLKF)
    bpp = mpart.reshape(NCORES * P, NBLKF, BLK).any(2).sum(1)
    assert epp.max() <= TOPK, f"edges/partition {epp.max()} > {TOPK}"
    assert bpp.max() <= TOPK, f"blocks/partition {bpp.max()} > {TOPK}"
    packs = (dst.astype(np.int64) * 64 + freq).reshape(NCORES * P, FREE)
    for prow in np.nonzero(epp > 1)[0]:
        vals = packs[prow][mpart[prow]]
        assert len(set(vals.tolist())) == len(vals), "duplicate (dst,freq) in partition"

    for c in range(NCORES):
        sl = slice(c * ESH, (c + 1) * ESH)
        packed = np.concatenate(
            [src[sl].reshape(NBLK, BLK), dst[sl].reshape(NBLK, BLK),
             freq[sl].reshape(NBLK, BLK)], axis=1)
        in_maps.append({
            "src": np.ascontiguousarray(src[sl].reshape(P, FREE)),
            "packed": np.ascontiguousarray(packed),
            "uidrep": uidrep, "uidpad": uidpad, "blkio": _BLKIO,
            "poi": poi, "uemb": uemb, "fcwt": fcwt, "fcb": fcb,
        })
    return path, in_maps


def kernel(**inputs):
    from concourse.bass_utils import run_bass_kernel_spmd

    path, in_maps = _make_in_maps(inputs)
    nc = _get_nc(path)
    res = run_bass_kernel_spmd(nc, in_maps, list(range(NCORES)))
    return np.asarray(res.results[0]["out"], dtype=np.float32)


# revision 12
# speedup vs baseline: 1.0556x; 1.0556x over previous
"""Distributed Trainium2 kernel for nn_AdjEmbeddings (gnn_message_passing).

Strategy (8 NeuronCores, edge-sharded):
  Only ~E/NUM_USERS (~32) of the 3.2M edges match the single user_idx, so the
  only tensor that needs a full read is edge_src.  Per core (400k-edge shard):
    1. Stream the src shard [128,3125] and compare against user_idx (DVE).
    2. Block-summarize matches (blocks of 25 edges) -> [128,125] indicator.
    3. Per-partition top-2 matched-block extraction (reduce_max + clear).
    4. Indirect-DMA gather the <=2 matched blocks/partition from a host-packed
       [16000, 75] (src|dst|freq) array; re-mask; per-partition top-2 matched
       edges; unpack (dst, freq) from a packed value dst*64+freq.
    5. Indirect-DMA gather the matched POI embedding rows; PE matmuls produce
       [1, 128+1] = (partial numerator | partial denominator).
    6. AllGather[8,129] across the 8 cores; every core reduces the partials
       locally and computes the epilogue (neigh = num/max(den,1), fc matmuls).
  Unmatched gather slots point out-of-bounds (skipped by the DMA) and carry
  weight 0, so they contribute nothing regardless of sim/HW fill behavior.
  NOTE: same-engine RAW hazards are real on this HW -- every dependent DVE op
  is serialized through the vq semaphore.
"""
import sys

if '/opt/trn_rl_repo' not in sys.path:
    sys.path.insert(0, '/opt/trn_rl_repo')

import numpy as np

NCORES = 8
E = 3_200_000
ESH = E // NCORES            # 400_000 edges per core
P = 128
FREE = ESH // P              # 3125
BLK = 25                     # edges per summary block
NBLKF = FREE // BLK          # 125 blocks per partition
NBLK = ESH // BLK            # 16000 blocks per core
TOPK = 2                     # matched blocks / edges extracted per partition
DIM = 128
NPOI = 50_000
NUSR = 100_000
BLK_SENT = 20_000            # > NBLK-1  -> OOB, skipped
POI_SENT = 60_000            # > NPOI-1  -> OOB, skipped
CLEAR = 1.0e7                # subtracted to clear extracted maxima
CH0 = 1575                   # stream chunk split (multiple of BLK)
NB0 = CH0 // BLK


def _build2():
    from concourse import bass, mybir
    from contextlib import ExitStack

    nc = bass.Bass(num_devices=NCORES)
    f32, i32 = mybir.dt.float32, mybir.dt.int32
    Alu = mybir.AluOpType
    X = mybir.AxisListType.X

    src_in = nc.declare_dram_parameter("src", [P, FREE], i32, isOutput=False)
    packed_in = nc.declare_dram_parameter("packed", [NBLK, 3 * BLK], i32, isOutput=False)
    uidrep_in = nc.declare_dram_parameter("uidrep", [P, 1], f32, isOutput=False)
    uidpad_in = nc.declare_dram_parameter("uidpad", [2, 1], i32, isOutput=False)
    blkio_in = nc.declare_dram_parameter("blkio", [P, NBLKF], f32, isOutput=False)
    poi_in = nc.declare_dram_parameter("poi", [NPOI, DIM], f32, isOutput=False)
    uemb_in = nc.declare_dram_parameter("uemb", [NUSR, DIM], f32, isOutput=False)
    fcwt_in = nc.declare_dram_parameter("fcwt", [2 * DIM, DIM], f32, isOutput=False)
    fcb_in = nc.declare_dram_parameter("fcb", [1, DIM], f32, isOutput=False)
    out_ext = nc.declare_dram_parameter("out", [1, DIM], f32, isOutput=True)

    cc_in = nc.dram_tensor("cc_in", [1, DIM + 1], f32)
    cc_ag = nc.dram_tensor("cc_ag", [NCORES, DIM + 1], f32, addr_space="Shared")

    es = ExitStack()

    def sb(name, shape, dt):
        return es.enter_context(nc.sbuf_tensor(name, shape, dt))

    def ps(name, shape):
        return es.enter_context(nc.psum_tensor(name, shape, f32))

    src_sb = sb('src_sb', [P, FREE], i32)
    mask_sb = sb('mask_sb', [P, FREE], f32)
    summ_sb = sb('summ_sb', [P, NBLKF], f32)
    blkio_sb = sb('blkio_sb', [P, NBLKF], f32)
    cand_sb = sb('cand_sb', [P, NBLKF], f32)
    eqb_sb = sb('eqb_sb', [P, NBLKF], f32)
    mtop_sb = sb('mtop_sb', [P, TOPK], f32)
    mm_sb = sb('mm_sb', [P, TOPK], f32)
    mtmp_sb = sb('mtmp_sb', [P, TOPK], f32)
    moff_sb = sb('moff_sb', [P, TOPK], i32)
    uid_sb = sb('uid_sb', [P, 1], f32)
    upad_sb = sb('upad_sb', [2, 1], i32)
    warmoff_sb = sb('warmoff_sb', [2, 1], i32)
    warm_sb = sb('warm_sb', [2, 3 * BLK], i32)
    g_sb = sb('g_sb', [P, 3 * BLK * TOPK], i32)
    mask2_sb = sb('mask2_sb', [P, BLK * TOPK], f32)
    dstf_sb = sb('dstf_sb', [P, BLK * TOPK], f32)
    freqf_sb = sb('freqf_sb', [P, BLK * TOPK], f32)
    packf_sb = sb('packf_sb', [P, BLK * TOPK], f32)
    cand2_sb = sb('cand2_sb', [P, BLK * TOPK], f32)
    eq2_sb = sb('eq2_sb', [P, BLK * TOPK], f32)
    etop_sb = sb('etop_sb', [P, TOPK], f32)
    em_sb = sb('em_sb', [P, TOPK], f32)
    ei_sb = sb('ei_sb', [P, TOPK], i32)
    dsti_sb = sb('dsti_sb', [P, TOPK], i32)
    freqi_sb = sb('freqi_sb', [P, TOPK], i32)
    frf_sb = sb('frf_sb', [P, TOPK], f32)
    wf_sb = sb('wf_sb', [P, TOPK], f32)
    dstf2_sb = sb('dstf2_sb', [P, TOPK], f32)
    dstt_sb = sb('dstt_sb', [P, TOPK], f32)
    dstoff_sb = sb('dstoff_sb', [P, TOPK], i32)
    paug_sb = sb('paug_sb', [P, TOPK * (DIM + 1)], f32)
    u2_sb = sb('u2_sb', [2, DIM], f32)
    ucol_sb = sb('ucol_sb', [P, 1], f32)
    ncol_sb = sb('ncol_sb', [P, 1], f32)
    nd_sb = sb('nd_sb', [1, DIM + 1], f32)
    cc8_sb = sb('cc8_sb', [NCORES, DIM + 1], f32)
    ones8_sb = sb('ones8_sb', [NCORES, 1], f32)
    saf_sb = sb('saf_sb', [1, 1], f32)
    rs_sb = sb('rs_sb', [1, 1], f32)
    t1_sb = sb('t1_sb', [1, DIM], f32)
    t2_sb = sb('t2_sb', [1, DIM], f32)
    fcw1_sb = sb('fcw1_sb', [P, DIM], f32)
    fcw2_sb = sb('fcw2_sb', [P, DIM], f32)
    fcb_sb = sb('fcb_sb', [1, DIM], f32)
    out_sb = sb('out_sb', [1, DIM], f32)
    ones11_sb = sb('ones11_sb', [1, 1], f32)

    psum_t1 = ps('psum_t1', [P, 1])
    psum_fc1 = ps('psum_fc1', [1, DIM])
    psum_nd = ps('psum_nd', [1, DIM + 1])
    psum_fc2 = ps('psum_fc2', [1, DIM])
    psum_nc = ps('psum_nc', [P, 1])
    psum_den = ps('psum_den', [1, 1])

    MK = {}
    with (
        nc.semaphore("vq") as vq,
        nc.semaphore("sS0") as sS0,
        nc.semaphore("sS1") as sS1,
        nc.semaphore("sC") as sC,
        nc.semaphore("sG") as sG,
        nc.semaphore("sPE") as sPE,
        nc.semaphore("sCCI") as sCCI,
        nc.semaphore("sCC") as sCC,
        nc.semaphore("sRED") as sRED,
        nc.Block() as block,
    ):
        @block.vector
        def _(vector):
            v = nc.vector
            nv = [0]

            def step(inst, wait=True):
                inst.then_inc(vq, 1)
                nv[0] += 1
                # serialize same-engine RAW hazards; independent ops may skip
                if wait:
                    vector.wait_ge(vq, nv[0])
                return nv[0]

            # independent setup (no internal deps -> no waits between them)
            step(v.memset(warmoff_sb[:], 0), wait=False)
            step(v.memset(ones11_sb[:], 1.0), wait=False)
            step(v.memset(g_sb[:], -1), wait=False)
            step(v.memset(paug_sb[:], 0.0), wait=False)
            step(v.memset(
                paug_sb[:].rearrange("p (j c) -> p j c", c=DIM + 1)[:, :, DIM:DIM + 1],
                1.0), wait=False)
            step(v.memset(ones8_sb[:], 1.0), wait=False)
            MK['setup'] = nv[0]
            vector.wait_ge(vq, nv[0])
            vector.wait_ge(sC, 80)
            vector.wait_ge(sS0, 32)     # blkio + src chunk0
            step(v.tensor_scalar(out=mask_sb[:, 0:CH0], in0=src_sb[:, 0:CH0],
                                 scalar1=uid_sb[:, :1], scalar2=None, op0=Alu.is_equal))
            step(v.tensor_reduce(
                out=summ_sb[:, 0:NB0],
                in_=mask_sb[:, 0:CH0].rearrange("p (b w) -> p b w", w=BLK),
                axis=X, op=Alu.max))
            vector.wait_ge(sS1, 16)
            step(v.tensor_scalar(out=mask_sb[:, CH0:FREE], in0=src_sb[:, CH0:FREE],
                                 scalar1=uid_sb[:, :1], scalar2=None, op0=Alu.is_equal))
            step(v.tensor_reduce(
                out=summ_sb[:, NB0:NBLKF],
                in_=mask_sb[:, CH0:FREE].rearrange("p (b w) -> p b w", w=BLK),
                axis=X, op=Alu.max))
            # cand = summ * (blkid+1) - 1   (blkio holds blkid+1)
            step(v.tensor_tensor(out=cand_sb[:], in0=summ_sb[:], in1=blkio_sb[:],
                                 op=Alu.mult))
            step(v.tensor_scalar_add(out=cand_sb[:], in0=cand_sb[:], scalar1=-1.0))
            # top-2 blocks per partition
            step(v.tensor_reduce(out=mtop_sb[:, 0:1], in_=cand_sb[:], axis=X, op=Alu.max))
            step(v.tensor_scalar(out=eqb_sb[:], in0=cand_sb[:],
                                 scalar1=mtop_sb[:, 0:1], scalar2=CLEAR,
                                 op0=Alu.is_equal, op1=Alu.mult))
            step(v.tensor_tensor(out=cand_sb[:], in0=cand_sb[:], in1=eqb_sb[:],
                                 op=Alu.subtract))
            step(v.tensor_reduce(out=mtop_sb[:, 1:2], in_=cand_sb[:], axis=X, op=Alu.max))
            # moff = matched ? blkid : BLK_SENT  (mtop holds blkid, or < 0)
            step(v.tensor_scalar(out=mm_sb[:], in0=mtop_sb[:], scalar1=0.0,
                                 scalar2=None, op0=Alu.is_ge))
            step(v.scalar_tensor_tensor(out=mtmp_sb[:], in0=mtop_sb[:],
                                        scalar=-float(BLK_SENT), in1=mm_sb[:],
                                        op0=Alu.add, op1=Alu.mult))
            step(v.tensor_scalar(out=moff_sb[:], in0=mtmp_sb[:],
                                 scalar1=float(BLK_SENT), scalar2=None, op0=Alu.add))
            MK['moff'] = nv[0]
            # ---- level 2: gathered blocks -> matched edges
            vector.wait_ge(sG, 64)          # warm(16)+u(16)+2 block gathers
            g3 = g_sb[:].rearrange("p (j c) -> p j c", c=3 * BLK)
            m23 = mask2_sb[:].rearrange("p (j c) -> p j c", c=BLK)
            d3 = dstf_sb[:].rearrange("p (j c) -> p j c", c=BLK)
            f3 = freqf_sb[:].rearrange("p (j c) -> p j c", c=BLK)
            step(v.tensor_scalar(out=m23, in0=g3[:, :, 0:BLK], scalar1=uid_sb[:, :1],
                                 scalar2=None, op0=Alu.is_equal), wait=False)
            step(v.tensor_copy(out=d3, in_=g3[:, :, BLK:2 * BLK]), wait=False)
            step(v.tensor_copy(out=f3, in_=g3[:, :, 2 * BLK:3 * BLK]))
            # packf = dst*64 + freq ; cand2 = (packf+1)*mask2 - 1
            step(v.scalar_tensor_tensor(out=packf_sb[:], in0=dstf_sb[:], scalar=64.0,
                                        in1=freqf_sb[:], op0=Alu.mult, op1=Alu.add))
            step(v.scalar_tensor_tensor(out=cand2_sb[:], in0=packf_sb[:], scalar=1.0,
                                        in1=mask2_sb[:], op0=Alu.add, op1=Alu.mult))
            step(v.tensor_scalar_add(out=cand2_sb[:], in0=cand2_sb[:], scalar1=-1.0))
            step(v.tensor_reduce(out=etop_sb[:, 0:1], in_=cand2_sb[:], axis=X, op=Alu.max))
            step(v.tensor_scalar(out=eq2_sb[:], in0=cand2_sb[:],
                                 scalar1=etop_sb[:, 0:1], scalar2=CLEAR,
                                 op0=Alu.is_equal, op1=Alu.mult))
            step(v.tensor_tensor(out=cand2_sb[:], in0=cand2_sb[:], in1=eq2_sb[:],
                                 op=Alu.subtract))
            step(v.tensor_reduce(out=etop_sb[:, 1:2], in_=cand2_sb[:], axis=X, op=Alu.max))
            # unpack: etop = dst*64+freq (>=64) matched, else < 0
            step(v.tensor_scalar(out=em_sb[:], in0=etop_sb[:], scalar1=0.0,
                                 scalar2=None, op0=Alu.is_ge))
            step(v.tensor_copy(out=ei_sb[:], in_=etop_sb[:]))
            step(v.tensor_scalar(out=dsti_sb[:], in0=ei_sb[:], scalar1=6, scalar2=None,
                                 op0=Alu.arith_shift_right), wait=False)
            step(v.tensor_scalar(out=freqi_sb[:], in0=ei_sb[:], scalar1=63, scalar2=None,
                                 op0=Alu.bitwise_and))
            step(v.tensor_copy(out=frf_sb[:], in_=freqi_sb[:]), wait=False)
            step(v.tensor_copy(out=dstf2_sb[:], in_=dsti_sb[:]))
            step(v.tensor_tensor(out=wf_sb[:], in0=frf_sb[:], in1=em_sb[:],
                                 op=Alu.mult), wait=False)
            step(v.scalar_tensor_tensor(out=dstt_sb[:], in0=dstf2_sb[:],
                                        scalar=-float(POI_SENT), in1=em_sb[:],
                                        op0=Alu.add, op1=Alu.mult))
            step(v.tensor_scalar(out=dstoff_sb[:], in0=dstt_sb[:],
                                 scalar1=float(POI_SENT), scalar2=None, op0=Alu.add))
            MK['dstoff'] = nv[0]
            # u column for the fc matmul (PE transposed it into psum_t1)
            vector.wait_ge(sPE, 1)
            step(v.tensor_copy(out=ucol_sb[:], in_=psum_t1[:]))
            MK['ucol'] = nv[0]
            # partials out for the collective
            vector.wait_ge(sPE, 3)
            step(v.tensor_copy(out=nd_sb[:], in_=psum_nd[:]))
            MK['nd'] = nv[0]
            # ---- after allgather: PE summed the partials into psum_nc/psum_den
            vector.wait_ge(sPE, 4)
            step(v.tensor_copy(out=ncol_sb[:], in_=psum_nc[:]), wait=False)
            # den is 0 (no matches anywhere -> num==0) or >= 1
            step(v.tensor_scalar(out=saf_sb[:], in0=psum_den[:], scalar1=1.0,
                                 scalar2=None, op0=Alu.max))
            MK['ncol'] = nv[0]
            step(v.reciprocal(out=rs_sb[:], in_=saf_sb[:]))
            MK['rs'] = nv[0]
            vector.wait_ge(sPE, 5)
            step(v.tensor_scalar(out=t1_sb[:], in0=psum_fc2[:], scalar1=rs_sb[0:1, :1],
                                 scalar2=None, op0=Alu.mult))
            step(v.tensor_tensor(out=t2_sb[:], in0=t1_sb[:], in1=psum_fc1[:], op=Alu.add))
            step(v.tensor_tensor(out=out_sb[:], in0=t2_sb[:], in1=fcb_sb[:], op=Alu.add))
            MK['out'] = nv[0]

        @block.sync
        def _(sync):
            sync.dma_start(out=uid_sb[:], in_=uidrep_in[:]).then_inc(sC, 16)
            sync.dma_start(out=upad_sb[:], in_=uidpad_in[:]).then_inc(sC, 16)
            sync.dma_start(out=fcb_sb[:], in_=fcb_in[:]).then_inc(sC, 16)
            sync.dma_start(out=fcw1_sb[:], in_=fcwt_in[0:DIM, :]).then_inc(sC, 16)
            sync.dma_start(out=fcw2_sb[:], in_=fcwt_in[DIM:2 * DIM, :]).then_inc(sC, 16)
            sync.dma_start(out=blkio_sb[:], in_=blkio_in[:]).then_inc(sS0, 16)
            sync.wait_ge(vq, MK['nd'])
            sync.dma_start(out=cc_in[:], in_=nd_sb[:]).then_inc(sCCI, 16)
            sync.wait_ge(sCC, 1)
            sync.dma_start(out=cc8_sb[:], in_=cc_ag[:]).then_inc(sRED, 16)
            sync.wait_ge(vq, MK['out'])
            sync.dma_start(out=out_ext[:], in_=out_sb[:]).then_inc(sS0, 16)

        @block.scalar
        def _(scalar):
            # second HWDGE ring: the big src stream
            scalar.dma_start(out=src_sb[:, 0:CH0], in_=src_in[:, 0:CH0]).then_inc(sS0, 16)
            scalar.dma_start(out=src_sb[:, CH0:FREE], in_=src_in[:, CH0:FREE]).then_inc(sS1, 16)

        @block.gpsimd
        def _(gpsimd):
            # warmup: pulls the indirect-DMA ucode load off the critical path
            gpsimd.wait_ge(vq, MK['setup'])
            gpsimd.indirect_dma_start(
                out=warm_sb[:], out_offset=None, in_=packed_in[:],
                in_offset=bass.IndirectOffsetOnAxis(ap=warmoff_sb[:, :1], axis=0),
                bounds_check=NBLK - 1, oob_is_err=False).then_inc(sG, 16)
            gpsimd.wait_ge(sC, 80)
            gpsimd.indirect_dma_start(
                out=u2_sb[:], out_offset=None, in_=uemb_in[:],
                in_offset=bass.IndirectOffsetOnAxis(ap=upad_sb[:, :1], axis=0),
                bounds_check=NUSR - 1, oob_is_err=False).then_inc(sG, 16)
            gpsimd.wait_ge(vq, MK['moff'])
            for j in range(TOPK):
                gpsimd.indirect_dma_start(
                    out=g_sb[:, j * 3 * BLK:(j + 1) * 3 * BLK], out_offset=None,
                    in_=packed_in[:],
                    in_offset=bass.IndirectOffsetOnAxis(ap=moff_sb[:, j:j + 1], axis=0),
                    bounds_check=NBLK - 1, oob_is_err=False).then_inc(sG, 16)
            gpsimd.wait_ge(vq, MK['dstoff'])
            for j in range(TOPK):
                gpsimd.indirect_dma_start(
                    out=paug_sb[:, j * (DIM + 1):j * (DIM + 1) + DIM], out_offset=None,
                    in_=poi_in[:],
                    in_offset=bass.IndirectOffsetOnAxis(ap=dstoff_sb[:, j:j + 1], axis=0),
                    bounds_check=NPOI - 1, oob_is_err=False).then_inc(sG, 16)
            gpsimd.wait_ge(sCCI, 16)
            gpsimd.collective_compute(
                "AllGather", mybir.AluOpType.bypass,
                replica_groups=[list(range(NCORES))],
                ins=[cc_in[:]], outs=[cc_ag[:]]).then_inc(sCC, 1)

        @block.tensor
        def _(tensor):
            tensor.wait_ge(sG, 32)            # u2 gathered
            tensor.wait_ge(vq, MK['setup'])   # ones11
            nc.tensor.transpose(out=psum_t1[:], in_=u2_sb[0:1, :],
                                identity=ones11_sb[:]).then_inc(sPE, 1)
            tensor.wait_ge(vq, MK['ucol'])
            tensor.wait_ge(sC, 80)
            nc.tensor.matmul(out=psum_fc1[:], lhsT=ucol_sb[:], rhs=fcw1_sb[:],
                             start=True, stop=True).then_inc(sPE, 1)
            tensor.wait_ge(vq, MK['dstoff'])
            tensor.wait_ge(sG, 96)            # poi gathered
            for j in range(TOPK):
                mmx = nc.tensor.matmul(
                    out=psum_nd[:], lhsT=wf_sb[:, j:j + 1],
                    rhs=paug_sb[:, j * (DIM + 1):(j + 1) * (DIM + 1)],
                    start=(j == 0), stop=(j == TOPK - 1))
            mmx.then_inc(sPE, 1)
            tensor.wait_ge(sRED, 16)
            nc.tensor.matmul(out=psum_nc[:], lhsT=cc8_sb[:, 0:DIM], rhs=ones8_sb[:],
                             start=True, stop=True)
            nc.tensor.matmul(out=psum_den[:], lhsT=cc8_sb[:, DIM:DIM + 1],
                             rhs=ones8_sb[:], start=True, stop=True).then_inc(sPE, 1)
            tensor.wait_ge(vq, MK['ncol'])
            nc.tensor.matmul(out=psum_fc2[:], lhsT=ncol_sb[:], rhs=fcw2_sb[:],
                             start=True, stop=True).then_inc(sPE, 1)

    es.close()
    return nc




def _build1():
    """TOPK=1 fast path: at most one matched edge per partition (host-checked).
    One fused pass per chunk: (src==uid)*edge_iota with accum_out giving the
    per-partition matched index directly (sum == the single match). Chunk 1
    runs on GpSimd concurrently with chunk 0 on DVE. All constants ride in
    two big DMAs (fiota_ext carries uid/pbase/fcb columns; fcw one load)."""
    from concourse import bass, mybir
    from contextlib import ExitStack

    nc = bass.Bass(num_devices=NCORES)
    f32, i32 = mybir.dt.float32, mybir.dt.int32
    Alu = mybir.AluOpType
    X = mybir.AxisListType.X
    ESENT = 1_000_000            # > ESH-1 -> OOB, skipped
    FEXT = FREE + 2 + DIM        # fiota | uid | pbase | fcb

    src_in = nc.declare_dram_parameter("src", [P, FREE], i32, isOutput=False)
    dfpk_in = nc.declare_dram_parameter("dfpk", [ESH, 2], i32, isOutput=False)
    fiota_in = nc.declare_dram_parameter("fiota", [P, FEXT], f32, isOutput=False)
    poi_in = nc.declare_dram_parameter("poi", [NPOI, DIM], f32, isOutput=False)
    uemb_in = nc.declare_dram_parameter("uemb", [NUSR, DIM], f32, isOutput=False)
    fcwt_in = nc.declare_dram_parameter("fcwt", [2 * DIM, DIM], f32, isOutput=False)
    out_ext = nc.declare_dram_parameter("out", [1, DIM], f32, isOutput=True)

    cc_in = nc.dram_tensor("cc_in", [1, DIM + 1], f32)
    cc_ag = nc.dram_tensor("cc_ag", [NCORES, DIM + 1], f32, addr_space="Shared")

    es = ExitStack()

    def sb(name, shape, dt):
        return es.enter_context(nc.sbuf_tensor(name, shape, dt))

    def ps(name, shape):
        return es.enter_context(nc.psum_tensor(name, shape, f32))

    src_sb = sb('src_sb', [P, FREE], i32)
    fiota_sb = sb('fiota_sb', [P, FEXT], f32)
    cand_sb = sb('cand_sb', [P, FREE], f32)
    ft0_sb = sb('ft0_sb', [P, 1], f32)
    ft1_sb = sb('ft1_sb', [P, 1], f32)
    ftop_sb = sb('ftop_sb', [P, 1], f32)
    m_sb = sb('m_sb', [P, 1], f32)
    t0_sb = sb('t0_sb', [P, 1], f32)
    t3_sb = sb('t3_sb', [P, 1], f32)
    eoff_sb = sb('eoff_sb', [P, 1], i32)
    g2_sb = sb('g2_sb', [P, 2], i32)
    frf_sb = sb('frf_sb', [P, 1], f32)
    wf_sb = sb('wf_sb', [P, 1], f32)
    dstf_sb = sb('dstf_sb', [P, 1], f32)
    dstt_sb = sb('dstt_sb', [P, 1], f32)
    poff_sb = sb('poff_sb', [P, 1], i32)
    paug_sb = sb('paug_sb', [P, DIM + 1], f32)
    upad_sb = sb('upad_sb', [2, 1], i32)
    warmoff_sb = sb('warmoff_sb', [2, 1], i32)
    warm_sb = sb('warm_sb', [2, 2], i32)
    u2_sb = sb('u2_sb', [2, DIM], f32)
    ucol_sb = sb('ucol_sb', [P, 1], f32)
    ncol_sb = sb('ncol_sb', [P, 1], f32)
    nd_sb = sb('nd_sb', [1, DIM + 1], f32)
    cc8_sb = sb('cc8_sb', [NCORES, DIM + 1], f32)
    ones8_sb = sb('ones8_sb', [NCORES, 1], f32)
    saf_sb = sb('saf_sb', [1, 1], f32)
    rs_sb = sb('rs_sb', [1, 1], f32)
    t1_sb = sb('t1_sb', [1, DIM], f32)
    t2_sb = sb('t2_sb', [1, DIM], f32)
    fcw_sb = sb('fcw_sb', [P, 2 * DIM], f32)
    out_sb = sb('out_sb', [1, DIM], f32)
    ones11_sb = sb('ones11_sb', [1, 1], f32)

    psum_t1 = ps('psum_t1', [P, 1])
    psum_fc1 = ps('psum_fc1', [1, DIM])
    psum_nd = ps('psum_nd', [1, DIM + 1])
    psum_fc2 = ps('psum_fc2', [1, DIM])
    psum_nc = ps('psum_nc', [P, 1])
    psum_den = ps('psum_den', [1, 1])

    uid_col = fiota_sb[:, FREE:FREE + 1]
    pbase_col = fiota_sb[:, FREE + 1:FREE + 2]
    fcb_row = fiota_sb[0:1, FREE + 2:FEXT]

    MK = {}
    with (
        nc.semaphore("vq") as vq,
        nc.semaphore("sS0") as sS0,
        nc.semaphore("sS1") as sS1,
        nc.semaphore("sC") as sC,
        nc.semaphore("sG") as sG,
        nc.semaphore("sGC") as sGC,
        nc.semaphore("sPE") as sPE,
        nc.semaphore("sCCI") as sCCI,
        nc.semaphore("sCC") as sCC,
        nc.semaphore("sRED") as sRED,
        nc.Block() as block,
    ):
        @block.vector
        def _(vector):
            v = nc.vector
            nv = [0]

            def step(inst, wait=True):
                inst.then_inc(vq, 1)
                nv[0] += 1
                if wait:
                    vector.wait_ge(vq, nv[0])
                return nv[0]

            step(v.memset(warmoff_sb[:], 0), wait=False)
            step(v.memset(ones11_sb[:], 1.0), wait=False)
            step(v.memset(g2_sb[:], 0), wait=False)
            step(v.memset(paug_sb[:, 0:DIM], 0.0), wait=False)
            step(v.memset(paug_sb[:, DIM:DIM + 1], 1.0), wait=False)
            step(v.memset(ones8_sb[:], 1.0), wait=False)
            MK['setup'] = nv[0]
            vector.wait_ge(vq, nv[0])
            vector.wait_ge(sS0, 32)     # fiota + src chunk0
            step(v.tensor_copy(out=upad_sb[:], in_=fiota_sb[0:2, FREE:FREE + 1]),
                 wait=False)
            MK['upad'] = nv[0]
            # cand = (src==uid) * (f+1); accum_out = row sum = matched f+1 (or 0)
            step(v.scalar_tensor_tensor(out=cand_sb[:, 0:CH0], in0=src_sb[:, 0:CH0],
                                        scalar=uid_col, in1=fiota_sb[:, 0:CH0],
                                        op0=Alu.is_equal, op1=Alu.mult,
                                        accum_out=ft0_sb[:]))
            vector.wait_ge(sS1, 16)
            step(v.scalar_tensor_tensor(out=cand_sb[:, CH0:FREE], in0=src_sb[:, CH0:FREE],
                                        scalar=uid_col, in1=fiota_sb[:, CH0:FREE],
                                        op0=Alu.is_equal, op1=Alu.mult,
                                        accum_out=ft1_sb[:]))
            step(v.tensor_tensor(out=ftop_sb[:], in0=ft0_sb[:], in1=ft1_sb[:],
                                 op=Alu.add))
            step(v.tensor_scalar(out=m_sb[:], in0=ftop_sb[:], scalar1=0.0,
                                 scalar2=None, op0=Alu.is_gt))
            step(v.scalar_tensor_tensor(out=t0_sb[:], in0=ftop_sb[:],
                                        scalar=-1.0 - ESENT, in1=pbase_col,
                                        op0=Alu.add, op1=Alu.add))
            step(v.tensor_tensor(out=t3_sb[:], in0=t0_sb[:], in1=m_sb[:], op=Alu.mult))
            step(v.tensor_scalar(out=eoff_sb[:], in0=t3_sb[:], scalar1=float(ESENT),
                                 scalar2=None, op0=Alu.add))
            MK['eoff'] = nv[0]
            # ---- after (dst,freq) gather
            vector.wait_ge(sG, 48)
            step(v.tensor_copy(out=frf_sb[:], in_=g2_sb[:, 1:2]), wait=False)
            step(v.tensor_copy(out=dstf_sb[:], in_=g2_sb[:, 0:1]))
            step(v.tensor_tensor(out=wf_sb[:], in0=frf_sb[:], in1=m_sb[:],
                                 op=Alu.mult), wait=False)
            step(v.scalar_tensor_tensor(out=dstt_sb[:], in0=dstf_sb[:],
                                        scalar=-float(POI_SENT), in1=m_sb[:],
                                        op0=Alu.add, op1=Alu.mult))
            step(v.tensor_scalar(out=poff_sb[:], in0=dstt_sb[:],
                                 scalar1=float(POI_SENT), scalar2=None, op0=Alu.add))
            MK['poff'] = nv[0]
            vector.wait_ge(sPE, 1)
            step(v.tensor_copy(out=ucol_sb[:], in_=psum_t1[:]))
            MK['ucol'] = nv[0]
            vector.wait_ge(sPE, 3)
            step(v.tensor_copy(out=nd_sb[:], in_=psum_nd[:]))
            MK['nd'] = nv[0]
            # ---- after allgather: PE summed partials into psum_nc/psum_den
            vector.wait_ge(sPE, 4)
            step(v.tensor_copy(out=ncol_sb[:], in_=psum_nc[:]), wait=False)
            step(v.tensor_scalar(out=saf_sb[:], in0=psum_den[:], scalar1=1.0,
                                 scalar2=None, op0=Alu.max))
            MK['ncol'] = nv[0]
            step(v.reciprocal(out=rs_sb[:], in_=saf_sb[:]))
            vector.wait_ge(sPE, 5)
            step(v.tensor_scalar(out=t1_sb[:], in0=psum_fc2[:], scalar1=rs_sb[0:1, :1],
                                 scalar2=None, op0=Alu.mult))
            step(v.tensor_tensor(out=t2_sb[:], in0=t1_sb[:], in1=psum_fc1[:], op=Alu.add))
            step(v.tensor_tensor(out=out_sb[:], in0=t2_sb[:], in1=fcb_row, op=Alu.add))
            MK['out'] = nv[0]

        @block.sync
        def _(sync):
            fcw_view = fcwt_in[:].rearrange("(j p) n -> p j n", p=P)
            sync.dma_start(out=fcw_sb[:], in_=fcw_view).then_inc(sC, 16)
            sync.wait_ge(vq, MK['nd'])
            sync.dma_start(out=cc_in[:], in_=nd_sb[:]).then_inc(sCCI, 16)
            sync.wait_ge(sCC, 1)
            sync.dma_start(out=cc8_sb[:], in_=cc_ag[:]).then_inc(sRED, 16)
            sync.wait_ge(vq, MK['out'])
            sync.dma_start(out=out_ext[:], in_=out_sb[:]).then_inc(sS0, 16)

        @block.scalar
        def _(scalar):
            scalar.dma_start(out=fiota_sb[:], in_=fiota_in[:]).then_inc(sS0, 16)
            scalar.dma_start(out=src_sb[:, 0:CH0], in_=src_in[:, 0:CH0]).then_inc(sS0, 16)
            scalar.dma_start(out=src_sb[:, CH0:FREE], in_=src_in[:, CH0:FREE]).then_inc(sS1, 16)

        @block.gpsimd
        def _(gpsimd):
            gpsimd.wait_ge(vq, MK['setup'])
            gpsimd.indirect_dma_start(
                out=warm_sb[:], out_offset=None, in_=dfpk_in[:],
                in_offset=bass.IndirectOffsetOnAxis(ap=warmoff_sb[:, :1], axis=0),
                bounds_check=ESH - 1, oob_is_err=False).then_inc(sG, 16)
            gpsimd.wait_ge(vq, MK['upad'])
            gpsimd.indirect_dma_start(
                out=u2_sb[:], out_offset=None, in_=uemb_in[:],
                in_offset=bass.IndirectOffsetOnAxis(ap=upad_sb[:, :1], axis=0),
                bounds_check=NUSR - 1, oob_is_err=False).then_inc(sG, 16)
            gpsimd.wait_ge(vq, MK['eoff'])
            gpsimd.indirect_dma_start(
                out=g2_sb[:], out_offset=None, in_=dfpk_in[:],
                in_offset=bass.IndirectOffsetOnAxis(ap=eoff_sb[:, :1], axis=0),
                bounds_check=ESH - 1, oob_is_err=False).then_inc(sG, 16)
            gpsimd.wait_ge(vq, MK['poff'])
            gpsimd.indirect_dma_start(
                out=paug_sb[:, 0:DIM], out_offset=None, in_=poi_in[:],
                in_offset=bass.IndirectOffsetOnAxis(ap=poff_sb[:, :1], axis=0),
                bounds_check=NPOI - 1, oob_is_err=False).then_inc(sG, 16)
            gpsimd.wait_ge(sCCI, 16)
            gpsimd.collective_compute(
                "AllGather", mybir.AluOpType.bypass,
                replica_groups=[list(range(NCORES))],
                ins=[cc_in[:]], outs=[cc_ag[:]]).then_inc(sCC, 1)

        @block.tensor
        def _(tensor):
            tensor.wait_ge(sG, 32)
            tensor.wait_ge(vq, MK['setup'])
            nc.tensor.transpose(out=psum_t1[:], in_=u2_sb[0:1, :],
                                identity=ones11_sb[:]).then_inc(sPE, 1)
            tensor.wait_ge(vq, MK['ucol'])
            tensor.wait_ge(sC, 16)
            nc.tensor.matmul(out=psum_fc1[:], lhsT=ucol_sb[:], rhs=fcw_sb[:, 0:DIM],
                             start=True, stop=True).then_inc(sPE, 1)
            tensor.wait_ge(vq, MK['poff'])
            tensor.wait_ge(sG, 64)
            nc.tensor.matmul(out=psum_nd[:], lhsT=wf_sb[:], rhs=paug_sb[:],
                             start=True, stop=True).then_inc(sPE, 1)
            tensor.wait_ge(sRED, 16)
            nc.tensor.matmul(out=psum_nc[:], lhsT=cc8_sb[:, 0:DIM], rhs=ones8_sb[:],
                             start=True, stop=True)
            nc.tensor.matmul(out=psum_den[:], lhsT=cc8_sb[:, DIM:DIM + 1],
                             rhs=ones8_sb[:], start=True, stop=True).then_inc(sPE, 1)
            tensor.wait_ge(vq, MK['ncol'])
            nc.tensor.matmul(out=psum_fc2[:], lhsT=ncol_sb[:], rhs=fcw_sb[:, DIM:2 * DIM],
                             start=True, stop=True).then_inc(sPE, 1)

    es.close()
    return nc


_BUILT = {}


def _get_nc(path=1):
    if path not in _BUILT:
        _BUILT[path] = _build1() if path == 1 else _build2()
    return _BUILT[path]


_BLKIO = None


def _make_in_maps(inputs):
    global _BLKIO
    user_idx = np.asarray(inputs["user_idx"]).astype(np.int32)
    poi = np.ascontiguousarray(np.asarray(inputs["poi_embeddings"], dtype=np.float32))
    src = np.asarray(inputs["edge_src"]).astype(np.int32)
    dst = np.asarray(inputs["edge_dst"]).astype(np.int32)
    freq = np.asarray(inputs["edge_freq"]).astype(np.int32)
    uemb = np.ascontiguousarray(np.asarray(inputs["user_emb"], dtype=np.float32))
    fc_w = np.asarray(inputs["fc_w"], dtype=np.float32)
    fc_b = np.asarray(inputs["fc_b"], dtype=np.float32)

    uid = int(user_idx[0])
    uidrep = np.full((P, 1), float(uid), np.float32)
    uidpad = np.full((2, 1), uid, np.int32)
    fcwt = np.ascontiguousarray(fc_w.T)
    fcb = fc_b.reshape(1, DIM)
    m = src == uid
    mpart = m.reshape(NCORES * P, FREE)
    epp = mpart.sum(1)
    path = 1 if epp.max() <= 1 else 2

    in_maps = []
    if path == 1:
        FEXT = FREE + 2 + DIM
        fiota = np.zeros((P, FEXT), np.float32)
        fiota[:, 0:FREE] = np.arange(1, FREE + 1, dtype=np.float32)
        fiota[:, FREE] = float(uid)
        fiota[:, FREE + 1] = np.arange(P, dtype=np.float32) * FREE
        fiota[0, FREE + 2:FEXT] = fc_b
        for c in range(NCORES):
            sl = slice(c * ESH, (c + 1) * ESH)
            dfpk = np.ascontiguousarray(
                np.stack([dst[sl], freq[sl]], axis=1))
            in_maps.append({
                "src": np.ascontiguousarray(src[sl].reshape(P, FREE)),
                "dfpk": dfpk, "fiota": fiota,
                "poi": poi, "uemb": uemb, "fcwt": fcwt,
            })
        return path, in_maps

    # TOPK=2 fallback: verify the static graph capacity (fail loudly
    # rather than return a wrong answer).
    if _BLKIO is None:
        _BLKIO = (np.arange(P * NBLKF, dtype=np.float32) + 1.0).reshape(P, NBLKF)
    bpp = mpart.reshape(NCORES * P, NBLKF, BLK).any(2).sum(1)
    assert epp.max() <= TOPK, f"edges/partition {epp.max()} > {TOPK}"
    assert bpp.max() <= TOPK, f"blocks/partition {bpp.max()} > {TOPK}"
    packs = (dst.astype(np.int64) * 64 + freq).reshape(NCORES * P, FREE)
    for prow in np.nonzero(epp > 1)[0]:
        vals = packs[prow][mpart[prow]]
        assert len(set(vals.tolist())) == len(vals), "duplicate (dst,freq) in partition"

    for c in range(NCORES):
        sl = slice(c * ESH, (c + 1) * ESH)
        packed = np.concatenate(
            [src[sl].reshape(NBLK, BLK), dst[sl].reshape(NBLK, BLK),
             freq[sl].reshape(NBLK, BLK)], axis=1)
        in_maps.append({
            "src": np.ascontiguousarray(src[sl].reshape(P, FREE)),
            "packed": np.ascontiguousarray(packed),
            "uidrep": uidrep, "uidpad": uidpad, "blkio": _BLKIO,
            "poi": poi, "uemb": uemb, "fcwt": fcwt, "fcb": fcb,
        })
    return path, in_maps


def kernel(**inputs):
    from concourse.bass_utils import run_bass_kernel_spmd

    path, in_maps = _make_in_maps(inputs)
    nc = _get_nc(path)
    res = run_bass_kernel_spmd(nc, in_maps, list(range(NCORES)))
    return np.asarray(res.results[0]["out"], dtype=np.float32)


# revision 15
# speedup vs baseline: 1.0834x; 1.0264x over previous
"""Distributed Trainium2 kernel for nn_AdjEmbeddings (gnn_message_passing).

Strategy (8 NeuronCores, edge-sharded):
  Only ~E/NUM_USERS (~32) of the 3.2M edges match the single user_idx, so the
  only tensor that needs a full read is edge_src.  Per core (400k-edge shard):
    1. Stream the src shard [128,3125] and compare against user_idx (DVE).
    2. Block-summarize matches (blocks of 25 edges) -> [128,125] indicator.
    3. Per-partition top-2 matched-block extraction (reduce_max + clear).
    4. Indirect-DMA gather the <=2 matched blocks/partition from a host-packed
       [16000, 75] (src|dst|freq) array; re-mask; per-partition top-2 matched
       edges; unpack (dst, freq) from a packed value dst*64+freq.
    5. Indirect-DMA gather the matched POI embedding rows; PE matmuls produce
       [1, 128+1] = (partial numerator | partial denominator).
    6. AllGather[8,129] across the 8 cores; every core reduces the partials
       locally and computes the epilogue (neigh = num/max(den,1), fc matmuls).
  Unmatched gather slots point out-of-bounds (skipped by the DMA) and carry
  weight 0, so they contribute nothing regardless of sim/HW fill behavior.
  NOTE: same-engine RAW hazards are real on this HW -- every dependent DVE op
  is serialized through the vq semaphore.
"""
import sys

if '/opt/trn_rl_repo' not in sys.path:
    sys.path.insert(0, '/opt/trn_rl_repo')

import numpy as np

NCORES = 8
E = 3_200_000
ESH = E // NCORES            # 400_000 edges per core
P = 128
FREE = ESH // P              # 3125
BLK = 25                     # edges per summary block
NBLKF = FREE // BLK          # 125 blocks per partition
NBLK = ESH // BLK            # 16000 blocks per core
TOPK = 2                     # matched blocks / edges extracted per partition
DIM = 128
NPOI = 50_000
NUSR = 100_000
BLK_SENT = 20_000            # > NBLK-1  -> OOB, skipped
POI_SENT = 60_000            # > NPOI-1  -> OOB, skipped
CLEAR = 1.0e7                # subtracted to clear extracted maxima
CH0 = 1575                   # stream chunk split (multiple of BLK)
NB0 = CH0 // BLK


def _build2():
    from concourse import bass, mybir
    from contextlib import ExitStack

    nc = bass.Bass(num_devices=NCORES)
    f32, i32 = mybir.dt.float32, mybir.dt.int32
    Alu = mybir.AluOpType
    X = mybir.AxisListType.X

    src_in = nc.declare_dram_parameter("src", [P, FREE], i32, isOutput=False)
    packed_in = nc.declare_dram_parameter("packed", [NBLK, 3 * BLK], i32, isOutput=False)
    uidrep_in = nc.declare_dram_parameter("uidrep", [P, 1], f32, isOutput=False)
    uidpad_in = nc.declare_dram_parameter("uidpad", [2, 1], i32, isOutput=False)
    blkio_in = nc.declare_dram_parameter("blkio", [P, NBLKF], f32, isOutput=False)
    poi_in = nc.declare_dram_parameter("poi", [NPOI, DIM], f32, isOutput=False)
    uemb_in = nc.declare_dram_parameter("uemb", [NUSR, DIM], f32, isOutput=False)
    fcwt_in = nc.declare_dram_parameter("fcwt", [2 * DIM, DIM], f32, isOutput=False)
    fcb_in = nc.declare_dram_parameter("fcb", [1, DIM], f32, isOutput=False)
    out_ext = nc.declare_dram_parameter("out", [1, DIM], f32, isOutput=True)

    cc_in = nc.dram_tensor("cc_in", [1, DIM + 1], f32)
    cc_ag = nc.dram_tensor("cc_ag", [NCORES, DIM + 1], f32, addr_space="Shared")

    es = ExitStack()

    def sb(name, shape, dt):
        return es.enter_context(nc.sbuf_tensor(name, shape, dt))

    def ps(name, shape):
        return es.enter_context(nc.psum_tensor(name, shape, f32))

    src_sb = sb('src_sb', [P, FREE], i32)
    mask_sb = sb('mask_sb', [P, FREE], f32)
    summ_sb = sb('summ_sb', [P, NBLKF], f32)
    blkio_sb = sb('blkio_sb', [P, NBLKF], f32)
    cand_sb = sb('cand_sb', [P, NBLKF], f32)
    eqb_sb = sb('eqb_sb', [P, NBLKF], f32)
    mtop_sb = sb('mtop_sb', [P, TOPK], f32)
    mm_sb = sb('mm_sb', [P, TOPK], f32)
    mtmp_sb = sb('mtmp_sb', [P, TOPK], f32)
    moff_sb = sb('moff_sb', [P, TOPK], i32)
    uid_sb = sb('uid_sb', [P, 1], f32)
    upad_sb = sb('upad_sb', [2, 1], i32)
    warmoff_sb = sb('warmoff_sb', [2, 1], i32)
    warm_sb = sb('warm_sb', [2, 3 * BLK], i32)
    g_sb = sb('g_sb', [P, 3 * BLK * TOPK], i32)
    mask2_sb = sb('mask2_sb', [P, BLK * TOPK], f32)
    dstf_sb = sb('dstf_sb', [P, BLK * TOPK], f32)
    freqf_sb = sb('freqf_sb', [P, BLK * TOPK], f32)
    packf_sb = sb('packf_sb', [P, BLK * TOPK], f32)
    cand2_sb = sb('cand2_sb', [P, BLK * TOPK], f32)
    eq2_sb = sb('eq2_sb', [P, BLK * TOPK], f32)
    etop_sb = sb('etop_sb', [P, TOPK], f32)
    em_sb = sb('em_sb', [P, TOPK], f32)
    ei_sb = sb('ei_sb', [P, TOPK], i32)
    dsti_sb = sb('dsti_sb', [P, TOPK], i32)
    freqi_sb = sb('freqi_sb', [P, TOPK], i32)
    frf_sb = sb('frf_sb', [P, TOPK], f32)
    wf_sb = sb('wf_sb', [P, TOPK], f32)
    dstf2_sb = sb('dstf2_sb', [P, TOPK], f32)
    dstt_sb = sb('dstt_sb', [P, TOPK], f32)
    dstoff_sb = sb('dstoff_sb', [P, TOPK], i32)
    paug_sb = sb('paug_sb', [P, TOPK * (DIM + 1)], f32)
    u2_sb = sb('u2_sb', [2, DIM], f32)
    ucol_sb = sb('ucol_sb', [P, 1], f32)
    ncol_sb = sb('ncol_sb', [P, 1], f32)
    nd_sb = sb('nd_sb', [1, DIM + 1], f32)
    cc8_sb = sb('cc8_sb', [NCORES, DIM + 1], f32)
    ones8_sb = sb('ones8_sb', [NCORES, 1], f32)
    saf_sb = sb('saf_sb', [1, 1], f32)
    rs_sb = sb('rs_sb', [1, 1], f32)
    t1_sb = sb('t1_sb', [1, DIM], f32)
    t2_sb = sb('t2_sb', [1, DIM], f32)
    fcw1_sb = sb('fcw1_sb', [P, DIM], f32)
    fcw2_sb = sb('fcw2_sb', [P, DIM], f32)
    fcb_sb = sb('fcb_sb', [1, DIM], f32)
    out_sb = sb('out_sb', [1, DIM], f32)
    ones11_sb = sb('ones11_sb', [1, 1], f32)

    psum_t1 = ps('psum_t1', [P, 1])
    psum_fc1 = ps('psum_fc1', [1, DIM])
    psum_nd = ps('psum_nd', [1, DIM + 1])
    psum_fc2 = ps('psum_fc2', [1, DIM])
    psum_nc = ps('psum_nc', [P, 1])
    psum_den = ps('psum_den', [1, 1])

    MK = {}
    with (
        nc.semaphore("vq") as vq,
        nc.semaphore("sS0") as sS0,
        nc.semaphore("sS1") as sS1,
        nc.semaphore("sC") as sC,
        nc.semaphore("sG") as sG,
        nc.semaphore("sPE") as sPE,
        nc.semaphore("sCCI") as sCCI,
        nc.semaphore("sCC") as sCC,
        nc.semaphore("sRED") as sRED,
        nc.Block() as block,
    ):
        @block.vector
        def _(vector):
            v = nc.vector
            nv = [0]

            def step(inst, wait=True):
                inst.then_inc(vq, 1)
                nv[0] += 1
                # serialize same-engine RAW hazards; independent ops may skip
                if wait:
                    vector.wait_ge(vq, nv[0])
                return nv[0]

            # independent setup (no internal deps -> no waits between them)
            step(v.memset(warmoff_sb[:], 0), wait=False)
            step(v.memset(ones11_sb[:], 1.0), wait=False)
            step(v.memset(g_sb[:], -1), wait=False)
            step(v.memset(paug_sb[:], 0.0), wait=False)
            step(v.memset(
                paug_sb[:].rearrange("p (j c) -> p j c", c=DIM + 1)[:, :, DIM:DIM + 1],
                1.0), wait=False)
            step(v.memset(ones8_sb[:], 1.0), wait=False)
            MK['setup'] = nv[0]
            vector.wait_ge(vq, nv[0])
            vector.wait_ge(sC, 80)
            vector.wait_ge(sS0, 32)     # blkio + src chunk0
            step(v.tensor_scalar(out=mask_sb[:, 0:CH0], in0=src_sb[:, 0:CH0],
                                 scalar1=uid_sb[:, :1], scalar2=None, op0=Alu.is_equal))
            step(v.tensor_reduce(
                out=summ_sb[:, 0:NB0],
                in_=mask_sb[:, 0:CH0].rearrange("p (b w) -> p b w", w=BLK),
                axis=X, op=Alu.max))
            vector.wait_ge(sS1, 16)
            step(v.tensor_scalar(out=mask_sb[:, CH0:FREE], in0=src_sb[:, CH0:FREE],
                                 scalar1=uid_sb[:, :1], scalar2=None, op0=Alu.is_equal))
            step(v.tensor_reduce(
                out=summ_sb[:, NB0:NBLKF],
                in_=mask_sb[:, CH0:FREE].rearrange("p (b w) -> p b w", w=BLK),
                axis=X, op=Alu.max))
            # cand = summ * (blkid+1) - 1   (blkio holds blkid+1)
            step(v.tensor_tensor(out=cand_sb[:], in0=summ_sb[:], in1=blkio_sb[:],
                                 op=Alu.mult))
            step(v.tensor_scalar_add(out=cand_sb[:], in0=cand_sb[:], scalar1=-1.0))
            # top-2 blocks per partition
            step(v.tensor_reduce(out=mtop_sb[:, 0:1], in_=cand_sb[:], axis=X, op=Alu.max))
            step(v.tensor_scalar(out=eqb_sb[:], in0=cand_sb[:],
                                 scalar1=mtop_sb[:, 0:1], scalar2=CLEAR,
                                 op0=Alu.is_equal, op1=Alu.mult))
            step(v.tensor_tensor(out=cand_sb[:], in0=cand_sb[:], in1=eqb_sb[:],
                                 op=Alu.subtract))
            step(v.tensor_reduce(out=mtop_sb[:, 1:2], in_=cand_sb[:], axis=X, op=Alu.max))
            # moff = matched ? blkid : BLK_SENT  (mtop holds blkid, or < 0)
            step(v.tensor_scalar(out=mm_sb[:], in0=mtop_sb[:], scalar1=0.0,
                                 scalar2=None, op0=Alu.is_ge))
            step(v.scalar_tensor_tensor(out=mtmp_sb[:], in0=mtop_sb[:],
                                        scalar=-float(BLK_SENT), in1=mm_sb[:],
                                        op0=Alu.add, op1=Alu.mult))
            step(v.tensor_scalar(out=moff_sb[:], in0=mtmp_sb[:],
                                 scalar1=float(BLK_SENT), scalar2=None, op0=Alu.add))
            MK['moff'] = nv[0]
            # ---- level 2: gathered blocks -> matched edges
            vector.wait_ge(sG, 64)          # warm(16)+u(16)+2 block gathers
            g3 = g_sb[:].rearrange("p (j c) -> p j c", c=3 * BLK)
            m23 = mask2_sb[:].rearrange("p (j c) -> p j c", c=BLK)
            d3 = dstf_sb[:].rearrange("p (j c) -> p j c", c=BLK)
            f3 = freqf_sb[:].rearrange("p (j c) -> p j c", c=BLK)
            step(v.tensor_scalar(out=m23, in0=g3[:, :, 0:BLK], scalar1=uid_sb[:, :1],
                                 scalar2=None, op0=Alu.is_equal), wait=False)
            step(v.tensor_copy(out=d3, in_=g3[:, :, BLK:2 * BLK]), wait=False)
            step(v.tensor_copy(out=f3, in_=g3[:, :, 2 * BLK:3 * BLK]))
            # packf = dst*64 + freq ; cand2 = (packf+1)*mask2 - 1
            step(v.scalar_tensor_tensor(out=packf_sb[:], in0=dstf_sb[:], scalar=64.0,
                                        in1=freqf_sb[:], op0=Alu.mult, op1=Alu.add))
            step(v.scalar_tensor_tensor(out=cand2_sb[:], in0=packf_sb[:], scalar=1.0,
                                        in1=mask2_sb[:], op0=Alu.add, op1=Alu.mult))
            step(v.tensor_scalar_add(out=cand2_sb[:], in0=cand2_sb[:], scalar1=-1.0))
            step(v.tensor_reduce(out=etop_sb[:, 0:1], in_=cand2_sb[:], axis=X, op=Alu.max))
            step(v.tensor_scalar(out=eq2_sb[:], in0=cand2_sb[:],
                                 scalar1=etop_sb[:, 0:1], scalar2=CLEAR,
                                 op0=Alu.is_equal, op1=Alu.mult))
            step(v.tensor_tensor(out=cand2_sb[:], in0=cand2_sb[:], in1=eq2_sb[:],
                                 op=Alu.subtract))
            step(v.tensor_reduce(out=etop_sb[:, 1:2], in_=cand2_sb[:], axis=X, op=Alu.max))
            # unpack: etop = dst*64+freq (>=64) matched, else < 0
            step(v.tensor_scalar(out=em_sb[:], in0=etop_sb[:], scalar1=0.0,
                                 scalar2=None, op0=Alu.is_ge))
            step(v.tensor_copy(out=ei_sb[:], in_=etop_sb[:]))
            step(v.tensor_scalar(out=dsti_sb[:], in0=ei_sb[:], scalar1=6, scalar2=None,
                                 op0=Alu.arith_shift_right), wait=False)
            step(v.tensor_scalar(out=freqi_sb[:], in0=ei_sb[:], scalar1=63, scalar2=None,
                                 op0=Alu.bitwise_and))
            step(v.tensor_copy(out=frf_sb[:], in_=freqi_sb[:]), wait=False)
            step(v.tensor_copy(out=dstf2_sb[:], in_=dsti_sb[:]))
            step(v.tensor_tensor(out=wf_sb[:], in0=frf_sb[:], in1=em_sb[:],
                                 op=Alu.mult), wait=False)
            step(v.scalar_tensor_tensor(out=dstt_sb[:], in0=dstf2_sb[:],
                                        scalar=-float(POI_SENT), in1=em_sb[:],
                                        op0=Alu.add, op1=Alu.mult))
            step(v.tensor_scalar(out=dstoff_sb[:], in0=dstt_sb[:],
                                 scalar1=float(POI_SENT), scalar2=None, op0=Alu.add))
            MK['dstoff'] = nv[0]
            # u column for the fc matmul (PE transposed it into psum_t1)
            vector.wait_ge(sPE, 1)
            step(v.tensor_copy(out=ucol_sb[:], in_=psum_t1[:]))
            MK['ucol'] = nv[0]
            # partials out for the collective

            # ---- after allgather: PE summed the partials into psum_nc/psum_den
            vector.wait_ge(sPE, 4)
            step(v.tensor_copy(out=ncol_sb[:], in_=psum_nc[:]), wait=False)
            # den is 0 (no matches anywhere -> num==0) or >= 1
            step(v.tensor_scalar(out=saf_sb[:], in0=psum_den[:], scalar1=1.0,
                                 scalar2=None, op0=Alu.max))
            MK['ncol'] = nv[0]
            step(v.reciprocal(out=rs_sb[:], in_=saf_sb[:]))
            MK['rs'] = nv[0]
            vector.wait_ge(sPE, 5)
            step(v.tensor_scalar(out=t1_sb[:], in0=psum_fc2[:], scalar1=rs_sb[0:1, :1],
                                 scalar2=None, op0=Alu.mult))
            step(v.tensor_tensor(out=t2_sb[:], in0=t1_sb[:], in1=psum_fc1[:], op=Alu.add))
            step(v.tensor_tensor(out=out_sb[:], in0=t2_sb[:], in1=fcb_sb[:], op=Alu.add))
            MK['out'] = nv[0]

        @block.sync
        def _(sync):
            sync.dma_start(out=uid_sb[:], in_=uidrep_in[:]).then_inc(sC, 16)
            sync.dma_start(out=upad_sb[:], in_=uidpad_in[:]).then_inc(sC, 16)
            sync.dma_start(out=fcb_sb[:], in_=fcb_in[:]).then_inc(sC, 16)
            sync.dma_start(out=fcw1_sb[:], in_=fcwt_in[0:DIM, :]).then_inc(sC, 16)
            sync.dma_start(out=fcw2_sb[:], in_=fcwt_in[DIM:2 * DIM, :]).then_inc(sC, 16)
            sync.dma_start(out=blkio_sb[:], in_=blkio_in[:]).then_inc(sS0, 16)
            sync.wait_ge(vq, MK['nd'])
            sync.dma_start(out=cc_in[:], in_=nd_sb[:]).then_inc(sCCI, 16)
            sync.wait_ge(sCC, 1)
            sync.dma_start(out=cc8_sb[:], in_=cc_ag[:]).then_inc(sRED, 16)
            sync.wait_ge(vq, MK['out'])
            sync.dma_start(out=out_ext[:], in_=out_sb[:]).then_inc(sS0, 16)

        @block.scalar
        def _(scalar):
            # second HWDGE ring: the big src stream
            scalar.dma_start(out=src_sb[:, 0:CH0], in_=src_in[:, 0:CH0]).then_inc(sS0, 16)
            scalar.dma_start(out=src_sb[:, CH0:FREE], in_=src_in[:, CH0:FREE]).then_inc(sS1, 16)

        @block.gpsimd
        def _(gpsimd):
            # warmup: pulls the indirect-DMA ucode load off the critical path
            gpsimd.wait_ge(vq, MK['setup'])
            gpsimd.indirect_dma_start(
                out=warm_sb[:], out_offset=None, in_=packed_in[:],
                in_offset=bass.IndirectOffsetOnAxis(ap=warmoff_sb[:, :1], axis=0),
                bounds_check=NBLK - 1, oob_is_err=False).then_inc(sG, 16)
            gpsimd.wait_ge(sC, 80)
            gpsimd.indirect_dma_start(
                out=u2_sb[:], out_offset=None, in_=uemb_in[:],
                in_offset=bass.IndirectOffsetOnAxis(ap=upad_sb[:, :1], axis=0),
                bounds_check=NUSR - 1, oob_is_err=False).then_inc(sG, 16)
            gpsimd.wait_ge(vq, MK['moff'])
            for j in range(TOPK):
                gpsimd.indirect_dma_start(
                    out=g_sb[:, j * 3 * BLK:(j + 1) * 3 * BLK], out_offset=None,
                    in_=packed_in[:],
                    in_offset=bass.IndirectOffsetOnAxis(ap=moff_sb[:, j:j + 1], axis=0),
                    bounds_check=NBLK - 1, oob_is_err=False).then_inc(sG, 16)
            gpsimd.wait_ge(vq, MK['dstoff'])
            for j in range(TOPK):
                gpsimd.indirect_dma_start(
                    out=paug_sb[:, j * (DIM + 1):j * (DIM + 1) + DIM], out_offset=None,
                    in_=poi_in[:],
                    in_offset=bass.IndirectOffsetOnAxis(ap=dstoff_sb[:, j:j + 1], axis=0),
                    bounds_check=NPOI - 1, oob_is_err=False).then_inc(sG, 16)
            gpsimd.wait_ge(sCCI, 16)
            gpsimd.collective_compute(
                "AllGather", mybir.AluOpType.bypass,
                replica_groups=[list(range(NCORES))],
                ins=[cc_in[:]], outs=[cc_ag[:]]).then_inc(sCC, 1)

        @block.tensor
        def _(tensor):
            tensor.wait_ge(sG, 32)            # u2 gathered
            tensor.wait_ge(vq, MK['setup'])   # ones11
            nc.tensor.transpose(out=psum_t1[:], in_=u2_sb[0:1, :],
                                identity=ones11_sb[:]).then_inc(sPE, 1)
            tensor.wait_ge(vq, MK['ucol'])
            tensor.wait_ge(sC, 80)
            nc.tensor.matmul(out=psum_fc1[:], lhsT=ucol_sb[:], rhs=fcw1_sb[:],
                             start=True, stop=True).then_inc(sPE, 1)
            tensor.wait_ge(vq, MK['dstoff'])
            tensor.wait_ge(sG, 96)            # poi gathered
            for j in range(TOPK):
                mmx = nc.tensor.matmul(
                    out=psum_nd[:], lhsT=wf_sb[:, j:j + 1],
                    rhs=paug_sb[:, j * (DIM + 1):(j + 1) * (DIM + 1)],
                    start=(j == 0), stop=(j == TOPK - 1))
            mmx.then_inc(sPE, 1)
            tensor.wait_ge(sRED, 16)
            nc.tensor.matmul(out=psum_nc[:], lhsT=cc8_sb[:, 0:DIM], rhs=ones8_sb[:],
                             start=True, stop=True)
            nc.tensor.matmul(out=psum_den[:], lhsT=cc8_sb[:, DIM:DIM + 1],
                             rhs=ones8_sb[:], start=True, stop=True).then_inc(sPE, 1)
            tensor.wait_ge(vq, MK['ncol'])
            nc.tensor.matmul(out=psum_fc2[:], lhsT=ncol_sb[:], rhs=fcw2_sb[:],
                             start=True, stop=True).then_inc(sPE, 1)

    es.close()
    return nc




def _build1():
    """TOPK=1 fast path: at most one matched edge per partition (host-checked).
    One fused pass per chunk: (src==uid)*edge_iota with accum_out giving the
    per-partition matched index directly (sum == the single match). Chunk 1
    runs on GpSimd concurrently with chunk 0 on DVE. All constants ride in
    two big DMAs (fiota_ext carries uid/pbase/fcb columns; fcw one load)."""
    from concourse import bass, mybir
    from contextlib import ExitStack

    nc = bass.Bass(num_devices=NCORES)
    f32, i32 = mybir.dt.float32, mybir.dt.int32
    Alu = mybir.AluOpType
    X = mybir.AxisListType.X
    ESENT = 1_000_000            # > ESH-1 -> OOB, skipped
    FEXT = FREE + 2 + DIM        # fiota | uid | pbase | fcb

    src_in = nc.declare_dram_parameter("src", [P, FREE], i32, isOutput=False)
    dfpk_in = nc.declare_dram_parameter("dfpk", [ESH, 2], i32, isOutput=False)
    fiota_in = nc.declare_dram_parameter("fiota", [P, FEXT], f32, isOutput=False)
    poi_in = nc.declare_dram_parameter("poi", [NPOI, DIM], f32, isOutput=False)
    uemb_in = nc.declare_dram_parameter("uemb", [NUSR, DIM], f32, isOutput=False)
    fcwt_in = nc.declare_dram_parameter("fcwt", [2 * DIM, DIM], f32, isOutput=False)
    out_ext = nc.declare_dram_parameter("out", [1, DIM], f32, isOutput=True)

    cc_in = nc.dram_tensor("cc_in", [1, DIM + 1], f32)
    cc_ag = nc.dram_tensor("cc_ag", [NCORES, DIM + 1], f32, addr_space="Shared")

    es = ExitStack()

    def sb(name, shape, dt):
        return es.enter_context(nc.sbuf_tensor(name, shape, dt))

    def ps(name, shape):
        return es.enter_context(nc.psum_tensor(name, shape, f32))

    src_sb = sb('src_sb', [P, FREE], i32)
    fiota_sb = sb('fiota_sb', [P, FEXT], f32)
    cand_sb = sb('cand_sb', [P, FREE], f32)
    ft0_sb = sb('ft0_sb', [P, 1], f32)
    ft1_sb = sb('ft1_sb', [P, 1], f32)
    ftop_sb = sb('ftop_sb', [P, 1], f32)
    m_sb = sb('m_sb', [P, 1], f32)
    t0_sb = sb('t0_sb', [P, 1], f32)
    t3_sb = sb('t3_sb', [P, 1], f32)
    eoff_sb = sb('eoff_sb', [P, 1], i32)
    g2_sb = sb('g2_sb', [P, 2], i32)
    frf_sb = sb('frf_sb', [P, 1], f32)
    wf_sb = sb('wf_sb', [P, 1], f32)
    dstf_sb = sb('dstf_sb', [P, 1], f32)
    dstt_sb = sb('dstt_sb', [P, 1], f32)
    poff_sb = sb('poff_sb', [P, 1], i32)
    paug_sb = sb('paug_sb', [P, DIM + 1], f32)
    upad_sb = sb('upad_sb', [2, 1], i32)
    warmoff_sb = sb('warmoff_sb', [2, 1], i32)
    warm_sb = sb('warm_sb', [2, 2], i32)
    u2_sb = sb('u2_sb', [2, DIM], f32)
    ucol_sb = sb('ucol_sb', [P, 1], f32)
    ncol_sb = sb('ncol_sb', [P, 1], f32)
    nd_sb = sb('nd_sb', [1, DIM + 1], f32)
    cc8_sb = sb('cc8_sb', [NCORES, DIM + 1], f32)
    ones8_sb = sb('ones8_sb', [NCORES, 1], f32)
    saf_sb = sb('saf_sb', [1, 1], f32)
    rs_sb = sb('rs_sb', [1, 1], f32)
    t1_sb = sb('t1_sb', [1, DIM], f32)
    t2_sb = sb('t2_sb', [1, DIM], f32)
    fcw_sb = sb('fcw_sb', [P, 2 * DIM], f32)
    out_sb = sb('out_sb', [1, DIM], f32)
    ones11_sb = sb('ones11_sb', [1, 1], f32)

    psum_t1 = ps('psum_t1', [P, 1])
    psum_fc1 = ps('psum_fc1', [1, DIM])
    psum_nd = ps('psum_nd', [1, DIM + 1])
    psum_fc2 = ps('psum_fc2', [1, DIM])
    psum_nc = ps('psum_nc', [P, 1])
    psum_den = ps('psum_den', [1, 1])

    uid_col = fiota_sb[:, FREE:FREE + 1]
    pbase_col = fiota_sb[:, FREE + 1:FREE + 2]
    fcb_row = fiota_sb[0:1, FREE + 2:FEXT]

    MK = {}
    with (
        nc.semaphore("vq") as vq,
        nc.semaphore("sS0") as sS0,
        nc.semaphore("sS1") as sS1,
        nc.semaphore("sC") as sC,
        nc.semaphore("sG") as sG,
        nc.semaphore("sGC") as sGC,
        nc.semaphore("sPE") as sPE,
        nc.semaphore("sCCI") as sCCI,
        nc.semaphore("sCC") as sCC,
        nc.semaphore("sRED") as sRED,
        nc.Block() as block,
    ):
        @block.vector
        def _(vector):
            v = nc.vector
            nv = [0]

            def step(inst, wait=True):
                inst.then_inc(vq, 1)
                nv[0] += 1
                if wait:
                    vector.wait_ge(vq, nv[0])
                return nv[0]

            step(v.memset(warmoff_sb[:], 0), wait=False)
            step(v.memset(ones11_sb[:], 1.0), wait=False)
            step(v.memset(g2_sb[:], 0), wait=False)
            step(v.memset(paug_sb[:, 0:DIM], 0.0), wait=False)
            step(v.memset(paug_sb[:, DIM:DIM + 1], 1.0), wait=False)
            step(v.memset(ones8_sb[:], 1.0), wait=False)
            MK['setup'] = nv[0]
            vector.wait_ge(vq, nv[0])
            vector.wait_ge(sS0, 32)     # fiota + src chunk0
            step(v.tensor_copy(out=upad_sb[:], in_=fiota_sb[0:2, FREE:FREE + 1]),
                 wait=False)
            MK['upad'] = nv[0]
            # cand = (src==uid) * (f+1); accum_out = row sum = matched f+1 (or 0)
            step(v.scalar_tensor_tensor(out=cand_sb[:, 0:CH0], in0=src_sb[:, 0:CH0],
                                        scalar=uid_col, in1=fiota_sb[:, 0:CH0],
                                        op0=Alu.is_equal, op1=Alu.mult,
                                        accum_out=ft0_sb[:]))
            vector.wait_ge(sS1, 16)
            step(v.scalar_tensor_tensor(out=cand_sb[:, CH0:FREE], in0=src_sb[:, CH0:FREE],
                                        scalar=uid_col, in1=fiota_sb[:, CH0:FREE],
                                        op0=Alu.is_equal, op1=Alu.mult,
                                        accum_out=ft1_sb[:]))
            step(v.tensor_tensor(out=ftop_sb[:], in0=ft0_sb[:], in1=ft1_sb[:],
                                 op=Alu.add))
            step(v.tensor_scalar(out=m_sb[:], in0=ftop_sb[:], scalar1=0.0,
                                 scalar2=None, op0=Alu.is_gt), wait=False)
            step(v.scalar_tensor_tensor(out=t0_sb[:], in0=ftop_sb[:],
                                        scalar=-1.0 - ESENT, in1=pbase_col,
                                        op0=Alu.add, op1=Alu.add))
            step(v.tensor_tensor(out=t3_sb[:], in0=t0_sb[:], in1=m_sb[:], op=Alu.mult))
            step(v.tensor_scalar(out=eoff_sb[:], in0=t3_sb[:], scalar1=float(ESENT),
                                 scalar2=None, op0=Alu.add))
            MK['eoff'] = nv[0]
            # ---- after (dst,freq) gather: w = freq * matched
            vector.wait_ge(sG, 48)
            step(v.tensor_copy(out=frf_sb[:], in_=g2_sb[:, 1:2]))
            step(v.tensor_tensor(out=wf_sb[:], in0=frf_sb[:], in1=m_sb[:],
                                 op=Alu.mult))
            MK['poff'] = nv[0]
            vector.wait_ge(sPE, 1)
            step(v.tensor_copy(out=ucol_sb[:], in_=psum_t1[:]))
            MK['ucol'] = nv[0]
            vector.wait_ge(sPE, 3)
            step(v.tensor_copy(out=nd_sb[:], in_=psum_nd[:]))
            MK['nd'] = nv[0]
            # ---- after allgather: PE summed partials into psum_nc/psum_den
            vector.wait_ge(sPE, 4)
            step(v.tensor_copy(out=ncol_sb[:], in_=psum_nc[:]), wait=False)
            step(v.tensor_scalar(out=saf_sb[:], in0=psum_den[:], scalar1=1.0,
                                 scalar2=None, op0=Alu.max))
            MK['ncol'] = nv[0]
            step(v.reciprocal(out=rs_sb[:], in_=saf_sb[:]))
            vector.wait_ge(sPE, 5)
            step(v.tensor_scalar(out=t1_sb[:], in0=psum_fc2[:], scalar1=rs_sb[0:1, :1],
                                 scalar2=None, op0=Alu.mult))
            step(v.tensor_tensor(out=t2_sb[:], in0=t1_sb[:], in1=psum_fc1[:], op=Alu.add))
            step(v.tensor_tensor(out=out_sb[:], in0=t2_sb[:], in1=fcb_row, op=Alu.add))
            MK['out'] = nv[0]

        @block.sync
        def _(sync):
            fcw_view = fcwt_in[:].rearrange("(j p) n -> p j n", p=P)
            sync.dma_start(out=fcw_sb[:], in_=fcw_view).then_inc(sC, 16)
            sync.wait_ge(vq, MK['nd'])
            sync.dma_start(out=cc_in[:], in_=nd_sb[:]).then_inc(sCCI, 16)
            sync.wait_ge(sCC, 1)
            sync.dma_start(out=cc8_sb[:], in_=cc_ag[:]).then_inc(sRED, 16)
            sync.wait_ge(vq, MK['out'])
            sync.dma_start(out=out_ext[:], in_=out_sb[:]).then_inc(sS0, 16)

        @block.scalar
        def _(scalar):
            scalar.dma_start(out=fiota_sb[:], in_=fiota_in[:]).then_inc(sS0, 16)
            scalar.dma_start(out=src_sb[:, 0:CH0], in_=src_in[:, 0:CH0]).then_inc(sS0, 16)
            scalar.dma_start(out=src_sb[:, CH0:FREE], in_=src_in[:, CH0:FREE]).then_inc(sS1, 16)

        @block.gpsimd
        def _(gpsimd):
            gpsimd.wait_ge(vq, MK['setup'])
            gpsimd.indirect_dma_start(
                out=warm_sb[:], out_offset=None, in_=dfpk_in[:],
                in_offset=bass.IndirectOffsetOnAxis(ap=warmoff_sb[:, :1], axis=0),
                bounds_check=ESH - 1, oob_is_err=False).then_inc(sG, 16)
            gpsimd.wait_ge(vq, MK['upad'])
            gpsimd.indirect_dma_start(
                out=u2_sb[:], out_offset=None, in_=uemb_in[:],
                in_offset=bass.IndirectOffsetOnAxis(ap=upad_sb[:, :1], axis=0),
                bounds_check=NUSR - 1, oob_is_err=False).then_inc(sG, 16)
            gpsimd.wait_ge(vq, MK['eoff'])
            gpsimd.indirect_dma_start(
                out=g2_sb[:], out_offset=None, in_=dfpk_in[:],
                in_offset=bass.IndirectOffsetOnAxis(ap=eoff_sb[:, :1], axis=0),
                bounds_check=ESH - 1, oob_is_err=False).then_inc(sG, 16)
            gpsimd.wait_ge(sG, 48)
            gpsimd.indirect_dma_start(
                out=paug_sb[:, 0:DIM], out_offset=None, in_=poi_in[:],
                in_offset=bass.IndirectOffsetOnAxis(ap=g2_sb[:, 0:1], axis=0),
                bounds_check=NPOI - 1, oob_is_err=False).then_inc(sG, 16)
            gpsimd.wait_ge(sCCI, 16)
            gpsimd.collective_compute(
                "AllGather", mybir.AluOpType.bypass,
                replica_groups=[list(range(NCORES))],
                ins=[cc_in[:]], outs=[cc_ag[:]]).then_inc(sCC, 1)

        @block.tensor
        def _(tensor):
            tensor.wait_ge(sG, 32)
            tensor.wait_ge(vq, MK['setup'])
            nc.tensor.transpose(out=psum_t1[:], in_=u2_sb[0:1, :],
                                identity=ones11_sb[:]).then_inc(sPE, 1)
            tensor.wait_ge(vq, MK['ucol'])
            tensor.wait_ge(sC, 16)
            nc.tensor.matmul(out=psum_fc1[:], lhsT=ucol_sb[:], rhs=fcw_sb[:, 0:DIM],
                             start=True, stop=True).then_inc(sPE, 1)
            tensor.wait_ge(vq, MK['poff'])
            tensor.wait_ge(sG, 64)
            nc.tensor.matmul(out=psum_nd[:], lhsT=wf_sb[:], rhs=paug_sb[:],
                             start=True, stop=True).then_inc(sPE, 1)
            tensor.wait_ge(sRED, 16)
            nc.tensor.matmul(out=psum_nc[:], lhsT=cc8_sb[:, 0:DIM], rhs=ones8_sb[:],
                             start=True, stop=True)
            nc.tensor.matmul(out=psum_den[:], lhsT=cc8_sb[:, DIM:DIM + 1],
                             rhs=ones8_sb[:], start=True, stop=True).then_inc(sPE, 1)
            tensor.wait_ge(vq, MK['ncol'])
            nc.tensor.matmul(out=psum_fc2[:], lhsT=ncol_sb[:], rhs=fcw_sb[:, DIM:2 * DIM],
                             start=True, stop=True).then_inc(sPE, 1)

    es.close()
    return nc


_BUILT = {}


def _get_nc(path=1):
    if path not in _BUILT:
        _BUILT[path] = _build1() if path == 1 else _build2()
    return _BUILT[path]


_BLKIO = None


def _make_in_maps(inputs):
    global _BLKIO
    user_idx = np.asarray(inputs["user_idx"]).astype(np.int32)
    poi = np.ascontiguousarray(np.asarray(inputs["poi_embeddings"], dtype=np.float32))
    src = np.asarray(inputs["edge_src"]).astype(np.int32)
    dst = np.asarray(inputs["edge_dst"]).astype(np.int32)
    freq = np.asarray(inputs["edge_freq"]).astype(np.int32)
    uemb = np.ascontiguousarray(np.asarray(inputs["user_emb"], dtype=np.float32))
    fc_w = np.asarray(inputs["fc_w"], dtype=np.float32)
    fc_b = np.asarray(inputs["fc_b"], dtype=np.float32)

    uid = int(user_idx[0])
    uidrep = np.full((P, 1), float(uid), np.float32)
    uidpad = np.full((2, 1), uid, np.int32)
    fcwt = np.ascontiguousarray(fc_w.T)
    fcb = fc_b.reshape(1, DIM)
    m = src == uid
    mpart = m.reshape(NCORES * P, FREE)
    epp = mpart.sum(1)
    path = 1 if epp.max() <= 1 else 2

    in_maps = []
    if path == 1:
        FEXT = FREE + 2 + DIM
        fiota = np.zeros((P, FEXT), np.float32)
        fiota[:, 0:FREE] = np.arange(1, FREE + 1, dtype=np.float32)
        fiota[:, FREE] = float(uid)
        fiota[:, FREE + 1] = np.arange(P, dtype=np.float32) * FREE
        fiota[0, FREE + 2:FEXT] = fc_b
        for c in range(NCORES):
            sl = slice(c * ESH, (c + 1) * ESH)
            dfpk = np.ascontiguousarray(
                np.stack([dst[sl], freq[sl]], axis=1))
            in_maps.append({
                "src": np.ascontiguousarray(src[sl].reshape(P, FREE)),
                "dfpk": dfpk, "fiota": fiota,
                "poi": poi, "uemb": uemb, "fcwt": fcwt,
            })
        return path, in_maps

    # TOPK=2 fallback: verify the static graph capacity (fail loudly
    # rather than return a wrong answer).
    if _BLKIO is None:
        _BLKIO = (np.arange(P * NBLKF, dtype=np.float32) + 1.0).reshape(P, NBLKF)
    bpp = mpart.reshape(NCORES * P, NBLKF, BLK).any(2).sum(1)
    assert epp.max() <= TOPK, f"edges/partition {epp.max()} > {TOPK}"
    assert bpp.max() <= TOPK, f"blocks/partition {bpp.max()} > {TOPK}"
    packs = (dst.astype(np.int64) * 64 + freq).reshape(NCORES * P, FREE)
    for prow in np.nonzero(epp > 1)[0]:
        vals = packs[prow][mpart[prow]]
        assert len(set(vals.tolist())) == len(vals), "duplicate (dst,freq) in partition"

    for c in range(NCORES):
        sl = slice(c * ESH, (c + 1) * ESH)
        packed = np.concatenate(
            [src[sl].reshape(NBLK, BLK), dst[sl].reshape(NBLK, BLK),
             freq[sl].reshape(NBLK, BLK)], axis=1)
        in_maps.append({
            "src": np.ascontiguousarray(src[sl].reshape(P, FREE)),
            "packed": np.ascontiguousarray(packed),
            "uidrep": uidrep, "uidpad": uidpad, "blkio": _BLKIO,
            "poi": poi, "uemb": uemb, "fcwt": fcwt, "fcb": fcb,
        })
    return path, in_maps


def kernel(**inputs):
    from concourse.bass_utils import run_bass_kernel_spmd

    path, in_maps = _make_in_maps(inputs)
    nc = _get_nc(path)
    res = run_bass_kernel_spmd(nc, in_maps, list(range(NCORES)))
    return np.asarray(res.results[0]["out"], dtype=np.float32)
